# revision 18
# baseline (speedup 1.0000x reference)
"""ASAP-GNN classifier on trn2 via Bass/Tile.

Architecture (v3): ONE fused device program, single launch, no host
round-trips inside the network. Everything — 3x (GraphConv + ASAPool
attention + LEConv fitness), top-k node selection, kNN graph
construction, gather-index construction, and the per-layer global max —
runs on one NeuronCore inside one NEFF built around hardware For_i
loops (small program => fast build + AOT).

Key ideas vs v2 (which used 3 programs and 6 launches):
  * top-k selection is replaced by an on-device threshold bisection on
    the fitness logits z: ~44 For_i iterations of count(z > t) -> exact
    top-k mask (no compaction; unselected nodes are masked with -3e30
    and keep their slots, all phases stay at 157 row-blocks).
  * the kNN graph is computed on device as a masked dense distance GEMM
    over all 20096 slots (mask folded into a 5th GEMM row), followed by
    two rounds of max8/max_index -> 16 candidates per node.
  * dma_gather index tiles for the kNN layers are built ON DEVICE from
    the candidate table via tensor-engine transposes (fp32) + int16
    cast + 8-group replication.
  * per-layer global max is masked-max over all slots, reduced on
    device via transposes; host only runs the final 1x1536 MLP.

Host does: layer-0 slot-table construction from edge_index, one batched
upload (~11MB), one launch, one 6KB download, final MLP.
"""

import math
import threading
import time
import numpy as np

N0 = 20000
IN_CH = 64
HID = 512
OUT = 10
L = 3
RATIO = 0.5

_f32 = np.float32

# ---- geometry constants ----
NB = 157                    # row blocks
R = NB * 128                # 20096
ROWS0 = R + 128             # x0 rows (sentinel row = R, zeros)
ROWS = 20608                # h/a/xw rows (>= CCOL pad, gather-safe)
SENT = R                    # sentinel row id
D0C_DEFAULT = 17            # layer-0 max in-degree (rebuilt if differs)
K1, K2 = 6, 8               # kNN k for layers 1, 2
CH = 40                     # candidate chunks of 512
CCOL = CH * 512             # 20480 candidate columns (padded)
BIS = 44                    # bisection iterations
KTGT = [10000.0, 5000.0, 2500.0]


# ----------------------------------------------------------------------------
# bass plumbing
# ----------------------------------------------------------------------------

_BASS = {}


def _get_bass():
    if not _BASS:
        import concourse.bass as bass
        import concourse.bacc as bacc
        import concourse.mybir as mybir
        from concourse.tile import TileContext
        from concourse.masks import make_identity
        from concourse.bass import ds
        from concourse import bass2jax
        import jax
        import jax.numpy as jnp
        try:
            jax.config.update("jax_compilation_cache_dir",
                              "/tmp/jax_nc_cache")
            jax.config.update("jax_persistent_cache_min_entry_size_bytes", -1)
            jax.config.update("jax_persistent_cache_min_compile_time_secs", 0.1)
        except Exception:
            pass
        bass2jax.install_neuronx_cc_hook()
        _BASS.update(bass=bass, bacc=bacc, mybir=mybir, TileContext=TileContext,
                     make_identity=make_identity, ds=ds, bass2jax=bass2jax,
                     jax=jax, jnp=jnp)
    return _BASS


class _Launcher:
    """Compiled 1-core bass program; inputs/outputs stay jax device arrays."""

    def __init__(self, nc):
        B = _get_bass()
        jax, jnp, mybir = B["jax"], B["jnp"], B["mybir"]
        bass2jax = B["bass2jax"]
        partition_name = (nc.partition_id_tensor.name
                          if nc.partition_id_tensor else None)
        in_names, in_avals, out_names, out_avals = [], [], [], []
        for alloc in nc.m.functions[0].allocations:
            if not isinstance(alloc, mybir.MemoryLocationSet):
                continue
            name = alloc.memorylocations[0].name
            if alloc.kind == "ExternalInput":
                if name != partition_name:
                    in_names.append(name)
                    in_avals.append(jax.ShapeDtypeStruct(
                        tuple(alloc.tensor_shape), mybir.dt.np(alloc.dtype)))
            elif alloc.kind == "ExternalOutput":
                out_names.append(name)
                out_avals.append(jax.core.ShapedArray(
                    tuple(alloc.tensor_shape), mybir.dt.np(alloc.dtype)))
        self.in_names = in_names
        self.in_avals = in_avals
        self.out_names = out_names
        self.out_avals = out_avals
        n_params = len(in_names)
        all_names = in_names + out_names + (
            [partition_name] if partition_name else [])
        donate = tuple(range(n_params, n_params + len(out_names)))

        def _body(*args):
            operands = list(args)
            if partition_name is not None:
                operands.append(bass2jax.partition_id_tensor())
            outs = bass2jax._bass_exec_p.bind(
                *operands, out_avals=tuple(out_avals),
                in_names=tuple(all_names), out_names=tuple(out_names),
                lowering_input_output_aliases=(),
                sim_require_finite=True, sim_require_nnan=True, nc=nc)
            return tuple(outs)

        self._jit = jax.jit(_body, donate_argnums=donate, keep_unused=True)
        self._compiled = None

    def warm(self):
        """AOT-compile the executable (no execution)."""
        B = _get_bass()
        jax = B["jax"]
        out_structs = [jax.ShapeDtypeStruct(av.shape, av.dtype)
                       for av in self.out_avals]
        self._compiled = self._jit.lower(*self.in_avals,
                                         *out_structs).compile()
        self._zeros = None
        try:
            jnp = B["jnp"]
            self._zeros = [jnp.zeros(av.shape, av.dtype).block_until_ready()
                           for av in self.out_avals]
        except Exception:
            pass
        return self

    def __call__(self, in_map):
        B = _get_bass()
        jnp = B["jnp"]
        args = [in_map[nm] for nm in self.in_names]
        zeros = getattr(self, "_zeros", None)
        if zeros is None:
            zeros = [jnp.zeros(av.shape, av.dtype) for av in self.out_avals]
        self._zeros = None    # donated; single-shot
        fn = self._compiled if self._compiled is not None else self._jit
        outs = fn(*args, *zeros)
        return dict(zip(self.out_names, outs))


# ----------------------------------------------------------------------------
# host helpers
# ----------------------------------------------------------------------------

def _idx_to_i16_tile(idx_list):
    """Compact dma_gather idx tile [16, S]: element m -> partition m%16,
    col m//16. Replicated across the 8 Q7 groups on device."""
    n = len(idx_list)
    S = (n + 15) // 16
    a = np.full((S, 16), -1, np.int16)
    a.reshape(-1)[:n] = idx_list.astype(np.int16)
    return np.ascontiguousarray(a.T)


def _slot_table(src, dst, nrows, D, sentinel):
    """[nrows, D] slot table: row i lists srcs of i's in-edges, sentinel pad."""
    deg = np.bincount(dst, minlength=nrows).astype(np.int64)
    order = np.argsort(dst, kind="stable")
    ss = src[order]
    dsrt = dst[order]
    starts = np.zeros(nrows + 1, np.int64)
    np.cumsum(deg, out=starts[1:])
    slot = np.arange(len(dsrt)) - starts[dsrt]
    tbl = np.full((nrows, D), sentinel, np.int64)
    tbl[dsrt, slot] = ss
    return tbl, deg


def _slotmajor_list(tbl):
    """[rows, D] -> block-slot-major gather list (per 128-block, slot-major)."""
    rows, D = tbl.shape
    nb = rows // 128
    return np.ascontiguousarray(
        tbl.reshape(nb, 128, D).transpose(0, 2, 1)).reshape(-1)


# ----------------------------------------------------------------------------
# blob layout (single batched upload)
# ----------------------------------------------------------------------------

def _layout(D0C):
    """name -> (word_offset, words, shape, kind). kind: f4 or i2 (bitcast)."""
    D0P = D0C + 1
    SP0 = NB * 8 * D0P
    specs = [
        ("x0", (ROWS0, IN_CH), "f4"),
        ("pidx0", (16, SP0), "i2"),
        ("invdeg", (R, 1), "f4"),
        ("cnt", (R, 1), "f4"),
        ("padmask", (128, NB), "f4"),
        ("selfidx", (16, NB * 8), "f4"),
        ("qc", (9, CCOL), "f4"),
        ("wxm", (128, HID), "f4"),
        ("wr1", (128, 4 * HID), "f4"),
        ("wl1", (128, 4 * HID), "f4"),
        ("wr2", (128, 4 * HID), "f4"),
        ("wl2", (128, 4 * HID), "f4"),
        ("vecs", (18, HID), "f4"),
        ("sc", (128, 16), "f4"),
    ]
    out = {}
    off = 0
    for name, shape, kind in specs:
        n = int(np.prod(shape))
        words = n // 2 if kind == "i2" else n
        assert words * (2 if kind == "i2" else 1) == n
        out[name] = (off, words, shape, kind)
        off += words
    return out, off


# ----------------------------------------------------------------------------
# program builder
# ----------------------------------------------------------------------------

def _tree_sum(nc, view, n):
    """In-place binary-tree reduce over slot axis; result lands in slot 0."""
    w = n
    while w > 1:
        h = w // 2
        nc.vector.tensor_add(view(0, h), view(0, h), view(h, h))
        if w % 2:
            nc.vector.tensor_add(view(0, 1), view(0, 1), view(w - 1, 1))
        w = h


def _tree_max(nc, gview, oview, n):
    """Max over n slots of g into out tile (slot tile of n//2 width)."""
    h = n // 2
    nc.vector.tensor_max(oview(0, h), gview(0, h), gview(h, h))
    if n % 2:
        nc.vector.tensor_max(oview(0, 1), oview(0, 1), gview(n - 1, 1))
    w = h
    while w > 1:
        h2 = w // 2
        nc.vector.tensor_max(oview(0, h2), oview(0, h2), oview(h2, h2))
        if w % 2:
            nc.vector.tensor_max(oview(0, 1), oview(0, 1), oview(w - 1, 1))
        w = h2


def _build_F(D0C):
    """The fully fused 3-layer program."""
    B = _get_bass()
    bacc, mybir, TileContext = B["bacc"], B["mybir"], B["TileContext"]
    ds, make_identity = B["ds"], B["make_identity"]
    dt = mybir.dt
    AX = mybir.AxisListType.X
    OP = mybir.AluOpType
    ACT = mybir.ActivationFunctionType
    D0P = D0C + 1
    SP0B = 8 * D0P
    SP0 = NB * SP0B

    lay, total_words = _layout(D0C)

    nc = bacc.Bacc("TRN2", target_bir_lowering=False)
    blob_d = nc.dram_tensor("blob", [1, total_words], dt.float32,
                            kind="ExternalInput")
    x0_d = nc.dram_tensor("x0", [ROWS0, IN_CH], dt.float32, kind="Internal")
    pidx0_d = nc.dram_tensor("pidx0", [16, SP0], dt.int16, kind="Internal")
    invdeg_d = nc.dram_tensor("invdeg", [R, 1], dt.float32, kind="Internal")
    cnt_d = nc.dram_tensor("cnt", [R, 1], dt.float32, kind="Internal")
    padmask_d = nc.dram_tensor("padmask", [128, NB], dt.float32,
                               kind="Internal")
    selfidx_d = nc.dram_tensor("selfidx", [16, NB * 8], dt.float32,
                               kind="Internal")
    qc_d = nc.dram_tensor("qc", [9, CCOL], dt.float32, kind="Internal")
    wxm_d = nc.dram_tensor("wxm", [128, HID], dt.float32, kind="Internal")
    wr1_d = nc.dram_tensor("wr1", [128, 4, HID], dt.float32, kind="Internal")
    wl1_d = nc.dram_tensor("wl1", [128, 4, HID], dt.float32, kind="Internal")
    wr2_d = nc.dram_tensor("wr2", [128, 4, HID], dt.float32, kind="Internal")
    wl2_d = nc.dram_tensor("wl2", [128, 4, HID], dt.float32, kind="Internal")
    vecs_d = nc.dram_tensor("vecs", [18, HID], dt.float32, kind="Internal")
    sc_d = nc.dram_tensor("sc", [128, 16], dt.float32, kind="Internal")
    _scatter_dst = {
        "x0": x0_d[:, :], "pidx0": pidx0_d[:, :], "invdeg": invdeg_d[:, :],
        "cnt": cnt_d[:, :], "padmask": padmask_d[:, :],
        "selfidx": selfidx_d[:, :], "qc": qc_d[:, :], "wxm": wxm_d[:, :],
        "wr1": wr1_d[:, :, :].rearrange("p a c -> p (a c)"),
        "wl1": wl1_d[:, :, :].rearrange("p a c -> p (a c)"),
        "wr2": wr2_d[:, :, :].rearrange("p a c -> p (a c)"),
        "wl2": wl2_d[:, :, :].rearrange("p a c -> p (a c)"),
        "vecs": vecs_d[:, :], "sc": sc_d[:, :],
    }

    xs_d = nc.dram_tensor("xs", [128, 12], dt.float32, kind="ExternalOutput")

    h_d = nc.dram_tensor("fh", [ROWS, 576], dt.float32, kind="Internal")
    a_d = nc.dram_tensor("fa", [ROWS, 64], dt.float32, kind="Internal")
    zb_d = nc.dram_tensor("zb", [R, 1], dt.float32, kind="Internal")
    z_d = nc.dram_tensor("z", [R, 1], dt.float32, kind="Internal")
    zm_d = nc.dram_tensor("zm", [1, R], dt.float32, kind="Internal")
    xn_d = nc.dram_tensor("xn", [R, HID], dt.float32, kind="Internal")
    xw_d = nc.dram_tensor("xw", [ROWS, HID], dt.float32, kind="Internal")
    knn_d = nc.dram_tensor("knn", [R, 16], dt.uint16, kind="Internal")
    msk_d = nc.dram_tensor("msk", [1, R], dt.float32, kind="Internal")
    pad_d = nc.dram_tensor("padc", [1, CCOL - R], dt.float32, kind="Internal")

    with TileContext(nc) as tc:
        with tc.tile_pool(name="const", bufs=1) as cpool:
            # scatter the blob to the per-tensor internal DRAM buffers
            for nm, (off, words, shape, kind) in lay.items():
                src = blob_d[0:1, off:off + words]
                if kind == "i2":
                    src = src.bitcast(dt.int16)
                src = src.rearrange("one (r c) -> r (c one)", r=shape[0])
                nc.sync.dma_start(_scatter_dst[nm], src)

            ident = cpool.tile([128, 128], dt.float32)
            make_identity(nc, ident[:])
            ones_r = cpool.tile([1, 128], dt.float32)
            nc.vector.memset(ones_r[:], 1.0)
            idbig = cpool.tile([128, 128], dt.float32)
            nc.vector.tensor_scalar_mul(idbig[:], ident[:], 1e30)
            padmask_sb = cpool.tile([128, NB], dt.float32)
            nc.sync.dma_start(padmask_sb[:], padmask_d[:, :])
            sc_sb = cpool.tile([128, 16], dt.float32)
            nc.sync.dma_start(sc_sb[:], sc_d[:, :])
            xs_sb = cpool.tile([128, 12], dt.float32)
            runmax = cpool.tile([128, HID], dt.float32)
            nc.vector.memset(runmax[:], -1e30)
            zm_sb = cpool.tile([128, NB], dt.float32)
            masks = [cpool.tile([128, NB], dt.float32, name="mask%d" % i)
                     for i in range(L)]
            tlos = [cpool.tile([1, 1], dt.float32, name="tlo%d" % i)
                    for i in range(L)]
            tlo128s = [cpool.tile([128, 1], dt.float32, name="tlo128%d" % i)
                       for i in range(L)]
            # bisection state
            lohi = cpool.tile([1, 2], dt.float32)
            tcur = cpool.tile([1, 1], dt.float32)
            cnts = cpool.tile([1, 1], dt.float32)
            isgt = cpool.tile([1, 1], dt.float32)
            d1 = cpool.tile([1, 1], dt.float32)
            cntp = cpool.tile([128, 1], dt.float32)
            mn = cpool.tile([128, 1], dt.float32)
            mx = cpool.tile([128, 1], dt.float32)
            t1r = cpool.tile([1, 128], dt.float32)
            ones_c = cpool.tile([128, 1], dt.float32)
            nc.vector.memset(ones_c[:], 1.0)
            # sentinel rows
            srow = cpool.tile([1, 576], dt.float32)
            nc.vector.memset(srow[:], 0.0)
            nc.vector.memset(srow[:, 512:513], -1e30)
            nc.sync.dma_start(h_d[SENT:SENT + 1, :], srow[:])
            nc.sync.dma_start(a_d[SENT:SENT + 1, :], srow[:, 0:64])

            def load_vecs(pool, li):
                t = {}
                for j, nm in enumerate(
                        ["br", "wq", "aw2", "l1w", "l2w", "l3w"]):
                    v = pool.tile([128, HID], dt.float32, tag="v_" + nm)
                    nc.sync.dma_start(
                        v[:], vecs_d[li * 6 + j:li * 6 + j + 1, :]
                        .to_broadcast([128, HID]))
                    t[nm] = v
                t["qb"] = sc_sb[:, li * 3 + 0:li * 3 + 1]
                t["l1b"] = sc_sb[:, li * 3 + 1:li * 3 + 2]
                t["l3b"] = sc_sb[:, li * 3 + 2:li * 3 + 3]
                return t

            def pool_fit_phases(wpool, pspool, idx_sb, idxoff, DP, V, li,
                                cnt_imm):
                """ASAPool + LEConv fitness over DP slots (incl self).

                idx_sb: [128, *] int16 gather tile; per-block window at
                idxoff(i) with 8*DP cols. cnt_imm: None -> load cnt_d.
                """
                SPB = 8 * DP

                def pool_body(i):
                    g = wpool.tile([128, DP, 576], dt.float32, tag="g")
                    nc.gpsimd.dma_gather(
                        out_ap=g[:], in_ap=h_d[:, :],
                        idxs_ap=idx_sb[:, idxoff(i)],
                        num_idxs=128 * DP, num_idxs_reg=128 * DP,
                        elem_size=576, single_packet=False)
                    xq = wpool.tile([128, DP // 2, HID], dt.float32, tag="xq")
                    _tree_max(nc,
                              lambda lo, c: g[:, lo:lo + c, 0:HID],
                              lambda lo, c: xq[:, lo:lo + c, :], DP)
                    tmp = wpool.tile([128, HID], dt.float32, tag="tmp")
                    nc.vector.tensor_mul(tmp[:], xq[:, 0, :], V["wq"][:])
                    qs = wpool.tile([128, 1], dt.float32, tag="qs")
                    nc.vector.tensor_reduce(qs[:], tmp[:], axis=AX, op=OP.add)
                    nc.vector.tensor_add(qs[:], qs[:], V["qb"])
                    sc = wpool.tile([128, DP], dt.float32, tag="sc")
                    jsv = g[:, :, 512:513].squeeze(2)
                    nc.vector.tensor_scalar_add(sc[:], jsv, qs[:])
                    sc2 = wpool.tile([128, DP], dt.float32, tag="sc2")
                    nc.vector.tensor_scalar_mul(sc2[:], sc[:], 0.2)
                    nc.vector.tensor_max(sc[:], sc[:], sc2[:])
                    m = wpool.tile([128, 1], dt.float32, tag="m")
                    nc.vector.tensor_reduce(m[:], sc[:], axis=AX, op=OP.max)
                    nc.vector.tensor_scalar(sc[:], sc[:], m[:], None,
                                            op0=OP.subtract)
                    nc.scalar.activation(sc[:], sc[:], ACT.Exp)
                    ssum = wpool.tile([128, 1], dt.float32, tag="ssum")
                    nc.vector.tensor_reduce(ssum[:], sc[:], axis=AX, op=OP.add)
                    rec = wpool.tile([128, 1], dt.float32, tag="rec")
                    nc.vector.reciprocal(rec[:], ssum[:])
                    nc.vector.tensor_scalar_mul(sc[:], sc[:], rec[:])
                    gh = g[:, :, 0:HID]
                    nc.vector.tensor_mul(
                        gh, gh, sc[:].unsqueeze(2).to_broadcast(
                            [128, DP, HID]))
                    _tree_sum(nc, lambda lo, c: g[:, lo:lo + c, 0:HID], DP)
                    xn = g[:, 0, 0:HID]
                    nc.sync.dma_start(xn_d[ds(i * 128, 128), :], xn)
                    nc.vector.tensor_mul(tmp[:], xn, V["l1w"][:])
                    av = wpool.tile([128, 1], dt.float32, tag="av")
                    nc.vector.tensor_reduce(av[:], tmp[:], axis=AX, op=OP.add)
                    nc.sync.dma_start(a_d[ds(i * 128, 128), 0:1], av[:])
                    nc.vector.tensor_mul(tmp[:], xn, V["l2w"][:])
                    bv = wpool.tile([128, 1], dt.float32, tag="bv")
                    nc.vector.tensor_reduce(bv[:], tmp[:], axis=AX, op=OP.add)
                    nc.vector.tensor_mul(tmp[:], xn, V["l3w"][:])
                    cv = wpool.tile([128, 1], dt.float32, tag="cv")
                    nc.vector.tensor_reduce(cv[:], tmp[:], axis=AX, op=OP.add)
                    zb = wpool.tile([128, 1], dt.float32, tag="zb")
                    lb1 = wpool.tile([128, 1], dt.float32, tag="lb1")
                    if cnt_imm is None:
                        ct = wpool.tile([128, 1], dt.float32, tag="ct")
                        nc.sync.dma_start(ct[:], cnt_d[ds(i * 128, 128), :])
                        nc.vector.tensor_mul(zb[:], ct[:], bv[:])
                        nc.vector.tensor_mul(lb1[:], ct[:], V["l1b"])
                    else:
                        nc.vector.tensor_scalar_mul(zb[:], bv[:], cnt_imm)
                        nc.vector.tensor_scalar_mul(lb1[:], V["l1b"], cnt_imm)
                    nc.vector.tensor_sub(zb[:], cv[:], zb[:])
                    nc.vector.tensor_add(zb[:], zb[:], V["l3b"])
                    nc.vector.tensor_add(zb[:], zb[:], lb1[:])
                    nc.sync.dma_start(zb_d[ds(i * 128, 128), :], zb[:])

                with tc.For_i(0, NB) as i:
                    pool_body(i)

                def fit_body(i):
                    ga = wpool.tile([128, DP, 64], dt.float32, tag="ga")
                    nc.gpsimd.dma_gather(
                        out_ap=ga[:], in_ap=a_d[:, :],
                        idxs_ap=idx_sb[:, idxoff(i)],
                        num_idxs=128 * DP, num_idxs_reg=128 * DP,
                        elem_size=64, single_packet=False)
                    zs = wpool.tile([128, 1], dt.float32, tag="zs")
                    nc.vector.tensor_reduce(zs[:], ga[:, :, 0:1].squeeze(2),
                                            axis=AX, op=OP.add)
                    zbl = wpool.tile([128, 1], dt.float32, tag="zbl")
                    nc.sync.dma_start(zbl[:], zb_d[ds(i * 128, 128), :])
                    nc.vector.tensor_add(zs[:], zs[:], zbl[:])
                    nc.sync.dma_start(z_d[ds(i * 128, 128), :], zs[:])

                with tc.For_i(0, NB) as i:
                    fit_body(i)

            def thresh_phase(li, prevmask):
                """zm = z + prevmask; bisect to top-KTGT threshold."""
                with tc.tile_pool(name="tps%d" % li, bufs=1,
                                  space="PSUM") as pst:
                    nc.sync.dma_start(
                        zm_sb[:],
                        z_d[:, :].rearrange("(b p) one -> p (b one)", p=128))
                    nc.vector.tensor_add(zm_sb[:], zm_sb[:], prevmask[:])
                    nc.sync.dma_start(
                        zm_d[0:1, :].rearrange("one (b p) -> p (b one)",
                                               p=128), zm_sb[:])
                    # active min/max for lo/hi
                    neg = cpool.tile([128, NB], dt.float32, tag="neg%d" % li)
                    nc.vector.tensor_scalar_mul(neg[:], prevmask[:], -1.0)
                    nc.vector.tensor_add(neg[:], neg[:], zm_sb[:])
                    nc.vector.tensor_reduce(mn[:], neg[:], axis=AX, op=OP.min)
                    nc.vector.tensor_reduce(mx[:], zm_sb[:], axis=AX,
                                            op=OP.max)
                    tpr = pst.tile([1, 128], dt.float32, tag="tpr")
                    nc.tensor.transpose(tpr[:], mn[:], ident[:])
                    nc.vector.tensor_copy(t1r[:], tpr[:])
                    nc.vector.tensor_reduce(lohi[:, 0:1], t1r[:], axis=AX,
                                            op=OP.min)
                    nc.tensor.transpose(tpr[:], mx[:], ident[:])
                    nc.vector.tensor_copy(t1r[:], tpr[:])
                    nc.vector.tensor_reduce(lohi[:, 1:2], t1r[:], axis=AX,
                                            op=OP.max)
                    ktgt = KTGT[li]
                    tbp = pst.tile([128, 1], dt.float32, tag="tbp")
                    cnt1 = pst.tile([1, 1], dt.float32, tag="cnt1")
                    with tc.For_i(0, BIS) as it:
                        nc.vector.tensor_add(tcur[:], lohi[:, 0:1],
                                             lohi[:, 1:2])
                        nc.vector.tensor_scalar_mul(tcur[:], tcur[:], 0.5)
                        nc.tensor.matmul(tbp[:], ones_r[:], tcur[:],
                                         start=True, stop=True)
                        tbs = cpool.tile([128, 1], dt.float32, tag="tbs")
                        nc.vector.tensor_copy(tbs[:], tbp[:])
                        cmp = cpool.tile([128, NB], dt.float32, tag="cmp")
                        nc.vector.tensor_scalar(cmp[:], zm_sb[:], tbs[:],
                                                None, op0=OP.is_gt)
                        nc.vector.tensor_reduce(cntp[:], cmp[:], axis=AX,
                                                op=OP.add)
                        nc.tensor.matmul(cnt1[:], cntp[:], ones_c[:],
                                         start=True, stop=True)
                        nc.vector.tensor_copy(cnts[:], cnt1[:])
                        nc.vector.tensor_scalar(isgt[:], cnts[:], ktgt - 0.5,
                                                None, op0=OP.is_gt)
                        nc.vector.tensor_sub(d1[:], tcur[:], lohi[:, 0:1])
                        nc.vector.tensor_mul(d1[:], d1[:], isgt[:])
                        nc.vector.tensor_add(lohi[:, 0:1], lohi[:, 0:1],
                                             d1[:])
                        nc.vector.tensor_sub(d1[:], lohi[:, 1:2], tcur[:])
                        nc.vector.tensor_mul(d1[:], d1[:], isgt[:])
                        nc.vector.tensor_add(lohi[:, 1:2], tcur[:], d1[:])
                    nc.vector.tensor_copy(tlos[li][:], lohi[:, 0:1])
                    nc.tensor.matmul(tbp[:], ones_r[:], tlos[li][:],
                                     start=True, stop=True)
                    nc.vector.tensor_copy(tlo128s[li][:], tbp[:])
                    # maskadd = (zm > tlo ? 0 : -3e30)
                    nc.vector.tensor_scalar(masks[li][:], zm_sb[:],
                                            tlo128s[li][:], None,
                                            op0=OP.is_gt)
                    nc.vector.tensor_scalar_add(masks[li][:], masks[li][:],
                                                -1.0)
                    nc.vector.tensor_scalar_mul(masks[li][:], masks[li][:],
                                                3e30)

            def xw_xs_phase(li, write_xw):
                """xw = xn*sigmoid(zm) (-> xw_d), masked running max -> xs."""
                with (
                    tc.tile_pool(name="wp_w%d" % li, bufs=2) as wp,
                    tc.tile_pool(name="psw%d" % li, bufs=1,
                                 space="PSUM") as psw,
                ):
                    def w_body(i):
                        fv = wp.tile([128, 1], dt.float32, tag="fv")
                        nc.scalar.activation(fv[:], zm_sb[:, ds(i, 1)],
                                             ACT.Sigmoid)
                        xnb = wp.tile([128, HID], dt.float32, tag="xnb")
                        nc.sync.dma_start(xnb[:], xn_d[ds(i * 128, 128), :])
                        xw = wp.tile([128, HID], dt.float32, tag="xw")
                        nc.vector.tensor_scalar_mul(xw[:], xnb[:], fv[:])
                        if write_xw:
                            nc.sync.dma_start(xw_d[ds(i * 128, 128), :],
                                              xw[:])
                        nc.vector.tensor_scalar_add(
                            xw[:], xw[:], masks[li][:, ds(i, 1)])
                        nc.vector.tensor_max(runmax[:], runmax[:], xw[:])

                    with tc.For_i(0, NB) as i:
                        w_body(i)
                    # reduce runmax across partitions into xs_sb cols
                    for c in range(4):
                        tp = psw.tile([128, 128], dt.float32,
                                      tag="tp%d" % (c % 2))
                        nc.tensor.transpose(tp[:],
                                            runmax[:, c * 128:(c + 1) * 128],
                                            ident[:])
                        nc.vector.tensor_reduce(
                            xs_sb[:, li * 4 + c:li * 4 + c + 1], tp[:],
                            axis=AX, op=OP.max)
                    nc.vector.memset(runmax[:], -1e30)

            def knn_phase(li):
                """Masked dense kNN sweep -> knn_d (16 candidates/node)."""
                with (
                    tc.tile_pool(name="wp_k%d" % li, bufs=2) as wp,
                    tc.tile_pool(name="psk%d" % li, bufs=2,
                                 space="PSUM") as psk,
                ):
                    cand = wp.tile([5, CCOL], dt.float32, tag="cand", bufs=1)
                    nc.sync.dma_start(cand[0:4, :], qc_d[5:9, :])
                    row = wp.tile([128, CCOL], dt.float32, tag="row", bufs=1)
                    # mask row: reuse row[0:1] as scratch, then bounce via
                    # DRAM for the partition shift 0 -> 4
                    zmr = row[0:1, 0:R]
                    nc.sync.dma_start(zmr, zm_d[:, :])
                    nc.vector.tensor_scalar(zmr, zmr,
                                            tlos[li][:], None, op0=OP.is_gt)
                    nc.vector.tensor_scalar_add(zmr, zmr, -1.0)
                    nc.vector.tensor_scalar_mul(zmr, zmr, 3e30)
                    nc.sync.dma_start(msk_d[:, :], zmr)
                    nc.sync.dma_start(cand[4:5, 0:R], msk_d[:, :])
                    if CCOL > R:
                        padc = wp.tile([1, CCOL - R], dt.float32, tag="padc")
                        nc.vector.memset(padc[:], -3e30)
                        nc.sync.dma_start(pad_d[:, :], padc[:])
                        nc.sync.dma_start(cand[4:5, R:CCOL], pad_d[:, :])

                    HCOL = CCOL // 2

                    def k_body(i):
                        qsb = wp.tile([5, 128], dt.float32, tag="qsb")
                        nc.sync.dma_start(qsb[:], qc_d[0:5, ds(i * 128, 128)])
                        for ch in range(CH):
                            dps = psk.tile([128, 512], dt.float32,
                                           tag="d%d" % (ch % 2))
                            nc.tensor.matmul(dps[:], qsb[:],
                                             cand[:, ch * 512:(ch + 1) * 512],
                                             start=True, stop=True)
                            nc.scalar.activation(
                                row[:, ch * 512:(ch + 1) * 512], dps[:],
                                ACT.Copy)
                        # self-exclusion on the diagonal block
                        nc.vector.tensor_sub(row[:, ds(i * 128, 128)],
                                             row[:, ds(i * 128, 128)],
                                             idbig[:])
                        # per-half top-8 (max8 input cap is 16384)
                        vab = wp.tile([128, 16], dt.float32, tag="vab")
                        iab = wp.tile([128, 16], dt.float32, tag="iab")
                        vA = wp.tile([128, 8], dt.float32, tag="vA")
                        iA = wp.tile([128, 8], dt.uint32, tag="iA")
                        nc.vector.max(out=vA[:], in_=row[:, 0:HCOL])
                        nc.vector.max_index(iA[:], vA[:], row[:, 0:HCOL])
                        nc.vector.tensor_copy(vab[:, 0:8], vA[:])
                        nc.vector.tensor_copy(iab[:, 0:8], iA[:])
                        vB = wp.tile([128, 8], dt.float32, tag="vB")
                        iB = wp.tile([128, 8], dt.uint32, tag="iB")
                        nc.vector.max(out=vB[:], in_=row[:, HCOL:CCOL])
                        nc.vector.max_index(iB[:], vB[:], row[:, HCOL:CCOL])
                        nc.vector.tensor_copy(vab[:, 8:16], vB[:])
                        nc.vector.tensor_copy(iab[:, 8:16], iB[:])
                        nc.vector.tensor_scalar_add(iab[:, 8:16],
                                                    iab[:, 8:16],
                                                    float(HCOL))
                        # iab1 = idx + 1 (0 must not survive the eq-mask max)
                        nc.vector.tensor_scalar_add(iab[:], iab[:], 1.0)
                        # merge the two sorted top-8 lists -> top-10 distinct
                        outf = wp.tile([128, 16], dt.float32, tag="outf")
                        mm = wp.tile([128, 1], dt.float32, tag="mm")
                        eq = wp.tile([128, 16], dt.float32, tag="eq")
                        tmq = wp.tile([128, 16], dt.float32, tag="tmq")
                        for s in range(10):
                            nc.vector.tensor_reduce(mm[:], vab[:], axis=AX,
                                                    op=OP.max)
                            nc.vector.tensor_scalar(eq[:], vab[:], mm[:],
                                                    None, op0=OP.is_equal)
                            nc.vector.tensor_mul(tmq[:], eq[:], iab[:])
                            nc.vector.tensor_reduce(outf[:, s:s + 1], tmq[:],
                                                    axis=AX, op=OP.max)
                            nc.vector.tensor_scalar_mul(tmq[:], eq[:], -1e31)
                            nc.vector.tensor_add(vab[:], vab[:], tmq[:])
                        nc.vector.tensor_scalar_add(outf[:, 0:10],
                                                    outf[:, 0:10], -1.0)
                        i16 = wp.tile([128, 16], dt.uint16, tag="i16")
                        nc.vector.tensor_copy(i16[:, 0:10], outf[:, 0:10])
                        nc.sync.dma_start(knn_d[ds(i * 128, 128), 0:10],
                                          i16[:, 0:10])

                    with tc.For_i(0, NB) as i:
                        k_body(i)

            def idx_build_phase(pool, li, D):
                """Build [128, NB*(D+1)*8] int16 gather tile from knn_d."""
                W = (D + 1) * 8
                idxg = pool.tile([128, NB * W], dt.int16, tag="idxg")
                with (
                    tc.tile_pool(name="ib%d" % li, bufs=2) as ib,
                    tc.tile_pool(name="psib%d" % li, bufs=2,
                                 space="PSUM") as psib,
                ):
                    stage = ib.tile([16, NB, D + 1, 8], dt.float32,
                                    tag="stage", bufs=1)
                    nc.sync.dma_start(
                        stage[:, :, D, :],
                        selfidx_d[:, :].rearrange("p (b c) -> p b c", b=NB))

                    def ib_body(i):
                        kb = ib.tile([128, 16], dt.uint16, tag="kb")
                        nc.sync.dma_start(kb[:], knn_d[ds(i * 128, 128), :])
                        kf = ib.tile([128, 16], dt.float32, tag="kf")
                        nc.vector.tensor_copy(kf[:], kb[:])
                        t1ps = psib.tile([16, 128], dt.float32, tag="t1ps")
                        nc.tensor.transpose(t1ps[:], kf[:], ident[:])
                        t1t = ib.tile([16, 128], dt.float32, tag="t1t")
                        nc.vector.tensor_copy(t1t[:], t1ps[:])
                        for c8 in range(8):
                            t2ps = psib.tile([16, 16], dt.float32,
                                             tag="t2_%d" % (c8 % 2))
                            nc.tensor.transpose(t2ps[:],
                                                t1t[:, c8 * 16:(c8 + 1) * 16],
                                                ident[0:16, 0:16])
                            nc.vector.tensor_copy(
                                stage[:, ds(i, 1), 0:D, c8].squeeze(1),
                                t2ps[:, 0:D])

                    with tc.For_i(0, NB) as i:
                        ib_body(i)
                    idx16 = ib.tile([16, NB * W], dt.int16, tag="idx16",
                                    bufs=1)
                    nc.vector.tensor_copy(
                        idx16[:],
                        stage[:].rearrange("p a b c -> p (a b c)"))
                    for g in range(8):
                        nc.sync.dma_start(idxg[g * 16:(g + 1) * 16, :],
                                          idx16[:])
                return idxg

            def conv_knn_phase(wpool, pspool, idxg, D, V, wr_t, wl_t):
                """GraphConv over the kNN graph (D neighbors + self slot)."""
                DP = D + 1
                WB = DP * 8

                def c_body(i):
                    g = wpool.tile([128, DP, HID], dt.float32, tag="cg")
                    nc.gpsimd.dma_gather(
                        out_ap=g[:], in_ap=xw_d[:, :],
                        idxs_ap=idxg[:, ds(i * WB, WB)],
                        num_idxs=128 * DP, num_idxs_reg=128 * DP,
                        elem_size=HID, single_packet=False)
                    _tree_sum(nc, lambda lo, c: g[:, lo:lo + c, :], D)
                    mean = wpool.tile([128, HID], dt.float32, tag="mean")
                    nc.vector.tensor_scalar_mul(mean[:], g[:, 0, :], 1.0 / D)
                    hps = pspool.tile([128, HID], dt.float32, tag="hps")
                    xt = wpool.tile([128, 4, 128], dt.float32, tag="xt")
                    mt = wpool.tile([128, 4, 128], dt.float32, tag="mt")
                    for c in range(4):
                        tp = pspool.tile([128, 128], dt.float32,
                                         tag="tp%d" % (c % 2))
                        nc.tensor.transpose(tp[:],
                                            g[:, D, c * 128:(c + 1) * 128],
                                            ident[:])
                        nc.vector.tensor_copy(xt[:, c, :], tp[:])
                        tp2 = pspool.tile([128, 128], dt.float32,
                                          tag="tq%d" % (c % 2))
                        nc.tensor.transpose(tp2[:],
                                            mean[:, c * 128:(c + 1) * 128],
                                            ident[:])
                        nc.vector.tensor_copy(mt[:, c, :], tp2[:])
                    for c in range(4):
                        nc.tensor.matmul(hps[:], xt[:, c, :], wl_t[:, c, :],
                                         start=(c == 0), stop=False)
                    for c in range(4):
                        nc.tensor.matmul(hps[:], mt[:, c, :], wr_t[:, c, :],
                                         start=False, stop=(c == 3))
                    hsb = wpool.tile([128, 576], dt.float32, tag="hsb")
                    nc.vector.tensor_add(hsb[:, 0:HID], hps[:], V["br"][:])
                    nc.vector.tensor_scalar_max(hsb[:, 0:HID], hsb[:, 0:HID],
                                                0.0)
                    tmp = wpool.tile([128, HID], dt.float32, tag="ctmp")
                    nc.vector.tensor_mul(tmp[:], hsb[:, 0:HID], V["aw2"][:])
                    nc.vector.tensor_reduce(hsb[:, 512:513], tmp[:], axis=AX,
                                            op=OP.add)
                    nc.sync.dma_start(h_d[ds(i * 128, 128), 0:513],
                                      hsb[:, 0:513])

                with tc.For_i(0, NB) as i:
                    c_body(i)

            # ================= layer 0 =================
            with tc.tile_pool(name="seg0", bufs=1) as seg0:
                pidx0_sb = seg0.tile([128, SP0], dt.int16, tag="pidx0")
                for g in range(8):
                    nc.sync.dma_start(pidx0_sb[g * 16:(g + 1) * 16, :],
                                      pidx0_d[:, :])
                V0 = load_vecs(seg0, 0)
                wxm_sb = seg0.tile([128, HID], dt.float32, tag="wxm")
                nc.sync.dma_start(wxm_sb[:], wxm_d[:, :])

                with (
                    tc.tile_pool(name="l0c", bufs=2) as wp0,
                    tc.tile_pool(name="ps0", bufs=2, space="PSUM") as ps0,
                ):
                    def conv0_body(i):
                        g = wp0.tile([128, D0P, IN_CH], dt.float32, tag="g0")
                        nc.gpsimd.dma_gather(
                            out_ap=g[:], in_ap=x0_d[:, :],
                            idxs_ap=pidx0_sb[:, ds(i * SP0B, SP0B)],
                            num_idxs=128 * D0P, num_idxs_reg=128 * D0P,
                            elem_size=IN_CH, single_packet=False)
                        _tree_sum(nc, lambda lo, c: g[:, lo:lo + c, :], D0P)
                        xm = wp0.tile([128, 128], dt.float32, tag="xm")
                        nc.sync.dma_start(xm[:, 0:IN_CH],
                                          x0_d[ds(i * 128, 128), :])
                        # neighbors-only sum = sum(all slots) - self
                        nc.vector.tensor_sub(g[:, 0, :], g[:, 0, :],
                                             xm[:, 0:IN_CH])
                        iv = wp0.tile([128, 1], dt.float32, tag="iv")
                        nc.sync.dma_start(iv[:],
                                          invdeg_d[ds(i * 128, 128), :])
                        nc.vector.tensor_scalar_mul(xm[:, IN_CH:2 * IN_CH],
                                                    g[:, 0, :], iv[:])
                        tp = ps0.tile([128, 128], dt.float32, tag="tp0")
                        nc.tensor.transpose(tp[:], xm[:], ident[:])
                        lhsT = wp0.tile([128, 128], dt.float32, tag="lhsT")
                        nc.vector.tensor_copy(lhsT[:], tp[:])
                        hps = ps0.tile([128, HID], dt.float32, tag="hps0")
                        nc.tensor.matmul(hps[:], lhsT[:], wxm_sb[:],
                                         start=True, stop=True)
                        hsb = wp0.tile([128, 576], dt.float32, tag="hsb0")
                        nc.vector.tensor_add(hsb[:, 0:HID], hps[:],
                                             V0["br"][:])
                        nc.vector.tensor_scalar_max(hsb[:, 0:HID],
                                                    hsb[:, 0:HID], 0.0)
                        tmp = wp0.tile([128, HID], dt.float32, tag="tmp0")
                        nc.vector.tensor_mul(tmp[:], hsb[:, 0:HID],
                                             V0["aw2"][:])
                        nc.vector.tensor_reduce(hsb[:, 512:513], tmp[:],
                                                axis=AX, op=OP.add)
                        nc.sync.dma_start(h_d[ds(i * 128, 128), 0:513],
                                          hsb[:, 0:513])

                    with tc.For_i(0, NB) as i:
                        conv0_body(i)
                with (
                    tc.tile_pool(name="l0p", bufs=2) as wp0p,
                    tc.tile_pool(name="ps0p", bufs=2, space="PSUM") as ps0p,
                ):
                    pool_fit_phases(wp0p, ps0p, pidx0_sb,
                                    lambda i: ds(i * SP0B, SP0B), D0P, V0, 0,
                                    None)
            thresh_phase(0, padmask_sb)
            xw_xs_phase(0, True)
            knn_phase(0)

            # ================= layer 1 =================
            with tc.tile_pool(name="seg1", bufs=1) as seg1:
                idxg1 = idx_build_phase(seg1, 1, K1)
                with (
                    tc.tile_pool(name="l1", bufs=2) as wp1,
                    tc.tile_pool(name="ps1", bufs=1, space="PSUM") as ps1,
                ):
                    V1 = load_vecs(wp1, 1)
                    wr1_sb = wp1.tile([128, 4, HID], dt.float32, tag="wr",
                                      bufs=1)
                    nc.sync.dma_start(wr1_sb[:], wr1_d[:, :, :])
                    wl1_sb = wp1.tile([128, 4, HID], dt.float32, tag="wl",
                                      bufs=1)
                    nc.sync.dma_start(wl1_sb[:], wl1_d[:, :, :])
                    conv_knn_phase(wp1, ps1, idxg1, K1, V1, wr1_sb, wl1_sb)
                    pool_fit_phases(wp1, ps1, idxg1,
                                    lambda i: ds(i * (K1 + 1) * 8,
                                                 (K1 + 1) * 8),
                                    K1 + 1, V1, 1, float(K1 + 1))
            thresh_phase(1, masks[0])
            xw_xs_phase(1, True)
            knn_phase(1)

            # ================= layer 2 =================
            with tc.tile_pool(name="seg2", bufs=1) as seg2:
                idxg2 = idx_build_phase(seg2, 2, K2)
                with (
                    tc.tile_pool(name="l2", bufs=2) as wp2,
                    tc.tile_pool(name="ps2", bufs=1, space="PSUM") as ps2,
                ):
                    V2 = load_vecs(wp2, 2)
                    wr2_sb = wp2.tile([128, 4, HID], dt.float32, tag="wr",
                                      bufs=1)
                    nc.sync.dma_start(wr2_sb[:], wr2_d[:, :, :])
                    wl2_sb = wp2.tile([128, 4, HID], dt.float32, tag="wl",
                                      bufs=1)
                    nc.sync.dma_start(wl2_sb[:], wl2_d[:, :, :])
                    conv_knn_phase(wp2, ps2, idxg2, K2, V2, wr2_sb, wl2_sb)
                    pool_fit_phases(wp2, ps2, idxg2,
                                    lambda i: ds(i * (K2 + 1) * 8,
                                                 (K2 + 1) * 8),
                                    K2 + 1, V2, 2, float(K2 + 1))
            thresh_phase(2, masks[1])
            xw_xs_phase(2, False)

            nc.sync.dma_start(xs_d[:, :], xs_sb[:])
    nc.compile()
    return nc


# ----------------------------------------------------------------------------
# build/compile management (import-time warm-up)
# ----------------------------------------------------------------------------

_RUNNERS = {}
_BUILD_LOCK = threading.Lock()
_BUILD_THREADS = []
_READY = {"F_%d" % D0C_DEFAULT: threading.Event()}


def _get_runner(name, builder):
    with _BUILD_LOCK:
        if name in _RUNNERS:
            return _RUNNERS[name]
    r = _Launcher(builder()).warm()
    with _BUILD_LOCK:
        _RUNNERS.setdefault(name, r)
    return _RUNNERS[name]


def _warm():
    try:
        jini = threading.Thread(target=lambda: _get_bass()["jax"].devices())
        jini.start()
        name = "F_%d" % D0C_DEFAULT
        l = _Launcher(_build_F(D0C_DEFAULT))
        l.warm()
        with _BUILD_LOCK:
            _RUNNERS.setdefault(name, l)
        jini.join()
    except Exception:  # pragma: no cover - fallback path handles
        import traceback
        traceback.print_exc()
    finally:
        for ev in _READY.values():
            ev.set()


_BUILD_THREADS.append(threading.Thread(target=_warm, daemon=True))
_BUILD_THREADS[-1].start()


# ----------------------------------------------------------------------------
# numpy fallback (used only if the device path fails)
# ----------------------------------------------------------------------------

def _np_reference(x, pos, src, dst, W):
    f = _f32
    n = N0
    xs = []
    for i in range(L):
        wr, br, wl = W["wr"][i], W["br"][i], W["wl"][i]
        agg = np.zeros((n, x.shape[1]), f)
        np.add.at(agg, dst, x[src])
        deg = np.bincount(dst, minlength=n).astype(f)
        mean = agg / np.maximum(deg, 1)[:, None]
        h = np.maximum(mean @ wr + br + x @ wl, 0).astype(f)
        sl = np.arange(n)
        s_ = np.concatenate([src, sl])
        d_ = np.concatenate([dst, sl])
        xj = h[s_]
        xq = np.full((n, HID), -np.inf, f)
        np.maximum.at(xq, d_, xj)
        xq = (xq @ W["lw"][i] + W["lb"][i]).astype(f)
        aw, ab = W["aw"][i], W["ab"][i]
        score = (xq[d_] @ aw[:HID] + xj @ aw[HID:] + ab).astype(f)
        score = np.where(score > 0, score, f(0.2) * score).astype(f)
        smax = np.full(n, -np.inf, f)
        np.maximum.at(smax, d_, score)
        ex = np.exp(score - smax[d_])
        ssum = np.zeros(n, f)
        np.add.at(ssum, d_, ex)
        att = (ex / ssum[d_]).astype(f)
        xn = np.zeros((n, HID), f)
        np.add.at(xn, d_, xj * att[:, None])
        a = xn @ W["l1w"][i] + W["l1b"][i]
        b = xn @ W["l2w"][i]
        agg2 = np.zeros(n, f)
        np.add.at(agg2, d_, (a[s_] - b[d_]).astype(f))
        z = (agg2 + xn @ W["l3w"][i] + W["l3b"][i]).astype(f)
        k_keep = int(math.ceil(RATIO * n))
        fit64 = 1.0 / (1.0 + np.exp(-z.astype(np.float64)))
        perm = np.argpartition(-fit64, k_keep - 1)[:k_keep]
        fv = fit64[perm].astype(f)
        x = (xn[perm] * fv[:, None]).astype(f)
        xs.append(x.max(0))
        pos = pos[perm]
        n = k_keep
        if i < L - 1:
            k = 6 + 2 * i
            sq = np.sum(pos * pos, -1)
            dist = sq[:, None] + sq[None, :] - 2 * (pos @ pos.T)
            np.fill_diagonal(dist, np.inf)
            idx = np.argpartition(dist, k, 1)[:, :k]
            srt = np.take_along_axis(dist, idx, 1).argsort(1, kind="stable")
            idx = np.take_along_axis(idx, srt, 1)
            dst = np.repeat(np.arange(n), k)
            src = idx.reshape(-1)
    return xs


# ----------------------------------------------------------------------------
# main kernel
# ----------------------------------------------------------------------------

_EXEC_NS = []


def kernel(x, pos, edge_index, conv0_wr, conv0_br, conv0_wl, conv_wr, conv_br,
           conv_wl, pool_lin_w, pool_lin_b, pool_att_w, pool_att_b, le1_w,
           le1_b, le2_w, le3_w, le3_b, lin1_w, lin1_b, lin2_w, lin2_b):
    t_start = time.perf_counter()
    _EXEC_NS.clear()
    x = np.asarray(x, _f32)
    pos = np.asarray(pos, _f32)
    ei = np.asarray(edge_index).astype(np.int64)

    W = {
        "wr": [np.asarray(conv0_wr, _f32)] + [np.asarray(conv_wr[i], _f32)
                                              for i in range(L - 1)],
        "br": [np.asarray(conv0_br, _f32)] + [np.asarray(conv_br[i], _f32)
                                              for i in range(L - 1)],
        "wl": [np.asarray(conv0_wl, _f32)] + [np.asarray(conv_wl[i], _f32)
                                              for i in range(L - 1)],
        "lw": [np.asarray(pool_lin_w[i], _f32) for i in range(L)],
        "lb": [np.asarray(pool_lin_b[i], _f32) for i in range(L)],
        "aw": [np.asarray(pool_att_w[i], _f32) for i in range(L)],
        "ab": [float(pool_att_b[i]) for i in range(L)],
        "l1w": [np.asarray(le1_w[i], _f32) for i in range(L)],
        "l1b": [float(le1_b[i]) for i in range(L)],
        "l2w": [np.asarray(le2_w[i], _f32) for i in range(L)],
        "l3w": [np.asarray(le3_w[i], _f32) for i in range(L)],
        "l3b": [float(le3_b[i]) for i in range(L)],
    }
    try:
        xs = _device_forward(x, pos, ei, W)
    except Exception:
        import traceback
        traceback.print_exc()
        print("kernel: device path failed; numpy fallback")
        xs = _np_reference(x, pos, ei[0], ei[1], W)

    hcat = np.concatenate(xs)[None, :].astype(_f32)
    h1 = np.maximum(hcat @ np.asarray(lin1_w, _f32) +
                    np.asarray(lin1_b, _f32), 0)
    out = (h1 @ np.asarray(lin2_w, _f32) + np.asarray(lin2_b, _f32))
    dt_ns = int((time.perf_counter() - t_start) * 1e9)
    _EXEC_NS.append(("kernel", dt_ns))
    return out.astype(_f32)


def _device_forward(x, pos, ei, W):
    _T0 = time.perf_counter()
    src, dst = ei[0], ei[1]

    # ---------------- host prep (pure numpy, overlaps warm) ----------------
    deg0 = np.bincount(dst, minlength=R).astype(np.int64)
    D0C = max(int(deg0.max()), 1)
    name = "F_%d" % D0C

    x0 = np.zeros((ROWS0, IN_CH), _f32)
    x0[:N0] = x
    tblC, _ = _slot_table(src, dst, R, D0C, SENT)
    tblP = np.concatenate(
        [np.arange(R, dtype=np.int64)[:, None], tblC], 1)
    tblP[N0:, 0] = SENT
    pidx0 = _idx_to_i16_tile(_slotmajor_list(tblP))
    invdeg0 = (1.0 / np.maximum(deg0, 1.0)).astype(_f32)[:, None]
    cnt0 = (deg0 + 1).astype(_f32)[:, None]
    padmask = np.zeros((128, NB), _f32)
    for j in range(N0, R):
        padmask[j % 128, j // 128] = -3e30
    selfidx = np.zeros((16, NB * 8), _f32)
    ar = np.arange(R).reshape(NB, 8, 16)           # [b, c8, p16]
    selfidx[:, :] = ar.transpose(2, 0, 1).reshape(16, NB * 8)
    sq = np.sum(pos * pos, 1, dtype=_f32)
    qc = np.zeros((9, CCOL), _f32)
    qc[0, :N0] = 2.0 * pos[:, 0]
    qc[1, :N0] = 2.0 * pos[:, 1]
    qc[2, :N0] = -1.0
    qc[3, :N0] = -sq
    qc[4, :N0] = 1.0
    qc[5, :N0] = pos[:, 0]
    qc[6, :N0] = pos[:, 1]
    qc[7, :N0] = sq
    qc[7, N0:] = 1e30
    qc[8, :] = 1.0
    wxm = np.zeros((128, HID), _f32)
    wxm[0:IN_CH] = W["wl"][0]
    wxm[IN_CH:2 * IN_CH] = W["wr"][0]
    vecs = np.zeros((18, HID), _f32)
    sc = np.zeros((128, 16), _f32)
    for li in range(L):
        lw, lb = W["lw"][li], W["lb"][li]
        aw, ab = W["aw"][li], W["ab"][li]
        wq = (lw @ aw[:HID]).astype(_f32)
        qb = float(lb @ aw[:HID] + ab)
        vecs[li * 6 + 0] = W["br"][li] if li == 0 else W["br"][li]
        vecs[li * 6 + 1] = wq
        vecs[li * 6 + 2] = aw[HID:]
        vecs[li * 6 + 3] = W["l1w"][li]
        vecs[li * 6 + 4] = W["l2w"][li]
        vecs[li * 6 + 5] = W["l3w"][li]
        sc[:, li * 3 + 0] = qb
        sc[:, li * 3 + 1] = W["l1b"][li]
        sc[:, li * 3 + 2] = W["l3b"][li]
    wpack = {}
    for li in (1, 2):
        wpack["wr%d" % li] = np.ascontiguousarray(
            W["wr"][li].reshape(4, 128, HID).transpose(1, 0, 2))
        wpack["wl%d" % li] = np.ascontiguousarray(
            W["wl"][li].reshape(4, 128, HID).transpose(1, 0, 2))

    host = {"x0": x0, "pidx0": pidx0, "invdeg": invdeg0, "cnt": cnt0,
            "padmask": padmask, "selfidx": selfidx, "qc": qc, "wxm": wxm,
            "wr1": wpack["wr1"], "wl1": wpack["wl1"], "wr2": wpack["wr2"],
            "wl2": wpack["wl2"], "vecs": vecs, "sc": sc}
    lay, total_words = _layout(D0C)
    blob = np.empty((1, total_words), _f32)
    for nm, (off, words, shape, kind) in lay.items():
        a = host[nm]
        if kind == "i2":
            blob[0, off:off + words] = np.ascontiguousarray(
                a).reshape(-1).view(_f32)
        else:
            blob[0, off:off + words] = a.reshape(-1)
    _EXEC_NS.append(("prep", int((time.perf_counter() - _T0) * 1e9)))

    # ---------------- upload (overlaps AOT warm in the bg thread) ----------
    t0 = time.perf_counter()
    B = _get_bass()
    jax = B["jax"]
    dev = jax.devices()[0]
    inF = {"blob": jax.device_put(blob, dev)}
    inF["blob"].block_until_ready()
    _EXEC_NS.append(("puts", int((time.perf_counter() - t0) * 1e9)))

    # ---------------- wait program, launch, download -----------------------
    t0 = time.perf_counter()
    ev = _READY.get(name)
    if ev is not None:
        ev.wait()
    Frun = _RUNNERS.get(name) or _get_runner(name, lambda: _build_F(D0C))
    _EXEC_NS.append(("warmjoin", int((time.perf_counter() - t0) * 1e9)))

    t0 = time.perf_counter()
    rF = Frun(inF)
    xs_t = np.asarray(rF["xs"])       # [128, 12]
    _EXEC_NS.append(("exec", int((time.perf_counter() - t0) * 1e9)))

    xs = []
    for li in range(L):
        v = np.empty(HID, _f32)
        for c in range(4):
            v[c * 128:(c + 1) * 128] = xs_t[:, li * 4 + c]
        xs.append(v)
    return xs


def total_exec_ns():
    return sum(v for k, v in _EXEC_NS if k == "kernel")


def exec_breakdown():
    return list(_EXEC_NS)


# revision 21
# speedup vs baseline: 1.9740x; 1.9740x over previous
"""ASAP-GNN classifier on trn2 via Bass/Tile.

Architecture (v3): ONE fused device program, single launch, no host
round-trips inside the network. Everything — 3x (GraphConv + ASAPool
attention + LEConv fitness), top-k node selection, kNN graph
construction, gather-index construction, and the per-layer global max —
runs on one NeuronCore inside one NEFF built around hardware For_i
loops (small program => fast build + AOT).

Key ideas vs v2 (which used 3 programs and 6 launches):
  * top-k selection is replaced by an on-device threshold bisection on
    the fitness logits z: ~44 For_i iterations of count(z > t) -> exact
    top-k mask (no compaction; unselected nodes are masked with -3e30
    and keep their slots, all phases stay at 157 row-blocks).
  * the kNN graph is computed on device as a masked dense distance GEMM
    over all 20096 slots (mask folded into a 5th GEMM row), followed by
    two rounds of max8/max_index -> 16 candidates per node.
  * dma_gather index tiles for the kNN layers are built ON DEVICE from
    the candidate table via tensor-engine transposes (fp32) + int16
    cast + 8-group replication.
  * per-layer global max is masked-max over all slots, reduced on
    device via transposes; host only runs the final 1x1536 MLP.

Host does: layer-0 slot-table construction from edge_index, one batched
upload (~11MB), one launch, one 6KB download, final MLP.
"""

import math
import threading
import time
import numpy as np

N0 = 20000
IN_CH = 64
HID = 512
OUT = 10
L = 3
RATIO = 0.5

_f32 = np.float32

# ---- geometry constants ----
NB = 157                    # row blocks
R = NB * 128                # 20096
ROWS0 = R + 128             # x0 rows (sentinel row = R, zeros)
ROWS = 20608                # h/a/xw rows (>= CCOL pad, gather-safe)
SENT = R                    # sentinel row id
D0C_DEFAULT = 17            # layer-0 max in-degree (rebuilt if differs)
K1, K2 = 6, 8               # kNN k for layers 1, 2
CH = 40                     # candidate chunks of 512
CCOL = CH * 512             # 20480 candidate columns (padded)
BIS = 44                    # bisection iterations
KTGT = [10000.0, 5000.0, 2500.0]


# ----------------------------------------------------------------------------
# bass plumbing
# ----------------------------------------------------------------------------

_BASS = {}


def _get_bass():
    if not _BASS:
        import concourse.bass as bass
        import concourse.bacc as bacc
        import concourse.mybir as mybir
        from concourse.tile import TileContext
        from concourse.masks import make_identity
        from concourse.bass import ds
        from concourse import bass2jax
        import jax
        import jax.numpy as jnp
        try:
            jax.config.update("jax_compilation_cache_dir",
                              "/tmp/jax_nc_cache")
            jax.config.update("jax_persistent_cache_min_entry_size_bytes", -1)
            jax.config.update("jax_persistent_cache_min_compile_time_secs", 0.1)
        except Exception:
            pass
        bass2jax.install_neuronx_cc_hook()
        _BASS.update(bass=bass, bacc=bacc, mybir=mybir, TileContext=TileContext,
                     make_identity=make_identity, ds=ds, bass2jax=bass2jax,
                     jax=jax, jnp=jnp)
    return _BASS


class _NcShim:
    """Stands in for a compiled Bacc during jax lowering: the lowering rule
    touches only to_json_bytes(), m.arch, has_collectives and
    target_bir_lowering."""

    target_bir_lowering = False

    def __init__(self, arch, has_collectives, json_bytes):
        import types
        self.has_collectives = has_collectives
        self._jb = json_bytes
        self.m = types.SimpleNamespace(arch=arch)

    def to_json_bytes(self):
        return self._jb


def _meta_from_nc(nc):
    B = _get_bass()
    mybir = B["mybir"]
    partition_name = (nc.partition_id_tensor.name
                      if nc.partition_id_tensor else None)
    in_names, in_specs, out_names, out_specs = [], [], [], []
    for alloc in nc.m.functions[0].allocations:
        if not isinstance(alloc, mybir.MemoryLocationSet):
            continue
        name = alloc.memorylocations[0].name
        if alloc.kind == "ExternalInput":
            if name != partition_name:
                in_names.append(name)
                in_specs.append((tuple(alloc.tensor_shape),
                                 np.dtype(mybir.dt.np(alloc.dtype)).str))
        elif alloc.kind == "ExternalOutput":
            out_names.append(name)
            out_specs.append((tuple(alloc.tensor_shape),
                              np.dtype(mybir.dt.np(alloc.dtype)).str))
    return {"in_names": in_names, "in_specs": in_specs,
            "out_names": out_names, "out_specs": out_specs,
            "partition_name": partition_name, "arch": nc.m.arch,
            "has_collectives": bool(nc.has_collectives),
            "json_bytes": nc.to_json_bytes(), "ncobj": nc}


class _Launcher:
    """Compiled 1-core bass program; inputs/outputs stay jax device arrays."""

    def __init__(self, meta):
        B = _get_bass()
        jax, jnp = B["jax"], B["jnp"]
        bass2jax = B["bass2jax"]
        ncobj = meta.get("ncobj")
        if ncobj is None:
            ncobj = _NcShim(meta["arch"], meta["has_collectives"],
                            meta["json_bytes"])
        partition_name = meta["partition_name"]
        self.in_names = list(meta["in_names"])
        self.in_avals = [jax.ShapeDtypeStruct(s, np.dtype(d))
                         for s, d in meta["in_specs"]]
        self.out_names = list(meta["out_names"])
        self.out_avals = [jax.core.ShapedArray(s, np.dtype(d))
                          for s, d in meta["out_specs"]]
        out_avals = self.out_avals
        out_names = self.out_names
        n_params = len(self.in_names)
        all_names = self.in_names + out_names + (
            [partition_name] if partition_name else [])
        donate = tuple(range(n_params, n_params + len(out_names)))

        def _body(*args):
            operands = list(args)
            if partition_name is not None:
                operands.append(bass2jax.partition_id_tensor())
            outs = bass2jax._bass_exec_p.bind(
                *operands, out_avals=tuple(out_avals),
                in_names=tuple(all_names), out_names=tuple(out_names),
                lowering_input_output_aliases=(),
                sim_require_finite=True, sim_require_nnan=True, nc=ncobj)
            return tuple(outs)

        self._jit = jax.jit(_body, donate_argnums=donate, keep_unused=True)
        self._compiled = None

    def warm(self):
        """AOT-compile the executable (no execution)."""
        B = _get_bass()
        jax = B["jax"]
        out_structs = [jax.ShapeDtypeStruct(av.shape, av.dtype)
                       for av in self.out_avals]
        self._compiled = self._jit.lower(*self.in_avals,
                                         *out_structs).compile()
        self._zeros = None
        try:
            jnp = B["jnp"]
            self._zeros = [jnp.zeros(av.shape, av.dtype).block_until_ready()
                           for av in self.out_avals]
        except Exception:
            pass
        return self

    def __call__(self, in_map):
        B = _get_bass()
        jnp = B["jnp"]
        args = [in_map[nm] for nm in self.in_names]
        zeros = getattr(self, "_zeros", None)
        if zeros is None:
            zeros = [jnp.zeros(av.shape, av.dtype) for av in self.out_avals]
        self._zeros = None    # donated; single-shot
        fn = self._compiled if self._compiled is not None else self._jit
        outs = fn(*args, *zeros)
        return dict(zip(self.out_names, outs))


# ----------------------------------------------------------------------------
# host helpers
# ----------------------------------------------------------------------------

def _idx_to_i16_tile(idx_list):
    """Compact dma_gather idx tile [16, S]: element m -> partition m%16,
    col m//16. Replicated across the 8 Q7 groups on device."""
    n = len(idx_list)
    S = (n + 15) // 16
    a = np.full((S, 16), -1, np.int16)
    a.reshape(-1)[:n] = idx_list.astype(np.int16)
    return np.ascontiguousarray(a.T)


def _slot_table(src, dst, nrows, D, sentinel):
    """[nrows, D] slot table: row i lists srcs of i's in-edges, sentinel pad."""
    deg = np.bincount(dst, minlength=nrows).astype(np.int64)
    order = np.argsort(dst, kind="stable")
    ss = src[order]
    dsrt = dst[order]
    starts = np.zeros(nrows + 1, np.int64)
    np.cumsum(deg, out=starts[1:])
    slot = np.arange(len(dsrt)) - starts[dsrt]
    tbl = np.full((nrows, D), sentinel, np.int64)
    tbl[dsrt, slot] = ss
    return tbl, deg


def _slotmajor_list(tbl):
    """[rows, D] -> block-slot-major gather list (per 128-block, slot-major)."""
    rows, D = tbl.shape
    nb = rows // 128
    return np.ascontiguousarray(
        tbl.reshape(nb, 128, D).transpose(0, 2, 1)).reshape(-1)


# ----------------------------------------------------------------------------
# blob layout (single batched upload)
# ----------------------------------------------------------------------------

def _layout(D0C):
    """name -> (word_offset, words, shape, kind). kind: f4 or i2 (bitcast)."""
    D0P = D0C + 1
    SP0 = NB * 8 * D0P
    specs = [
        ("x0", (ROWS0, IN_CH), "f4"),
        ("pidx0", (16, SP0), "i2"),
        ("invdeg", (R, 1), "f4"),
        ("cnt", (R, 1), "f4"),
        ("padmask", (128, NB), "f4"),
        ("selfidx", (16, NB * 8), "f4"),
        ("qc", (9, CCOL), "f4"),
        ("wxm", (128, HID), "f4"),
        ("wr1", (128, 4 * HID), "f4"),
        ("wl1", (128, 4 * HID), "f4"),
        ("wr2", (128, 4 * HID), "f4"),
        ("wl2", (128, 4 * HID), "f4"),
        ("vecs", (18, HID), "f4"),
        ("sc", (128, 16), "f4"),
    ]
    out = {}
    off = 0
    for name, shape, kind in specs:
        n = int(np.prod(shape))
        words = n // 2 if kind == "i2" else n
        assert words * (2 if kind == "i2" else 1) == n
        out[name] = (off, words, shape, kind)
        off += words
    return out, off


# ----------------------------------------------------------------------------
# program builder
# ----------------------------------------------------------------------------

def _tree_sum(nc, view, n):
    """In-place binary-tree reduce over slot axis; result lands in slot 0."""
    w = n
    while w > 1:
        h = w // 2
        nc.vector.tensor_add(view(0, h), view(0, h), view(h, h))
        if w % 2:
            nc.vector.tensor_add(view(0, 1), view(0, 1), view(w - 1, 1))
        w = h


def _tree_max(nc, gview, oview, n):
    """Max over n slots of g into out tile (slot tile of n//2 width)."""
    h = n // 2
    nc.vector.tensor_max(oview(0, h), gview(0, h), gview(h, h))
    if n % 2:
        nc.vector.tensor_max(oview(0, 1), oview(0, 1), gview(n - 1, 1))
    w = h
    while w > 1:
        h2 = w // 2
        nc.vector.tensor_max(oview(0, h2), oview(0, h2), oview(h2, h2))
        if w % 2:
            nc.vector.tensor_max(oview(0, 1), oview(0, 1), oview(w - 1, 1))
        w = h2


def _build_F(D0C):
    """The fully fused 3-layer program."""
    B = _get_bass()
    bacc, mybir, TileContext = B["bacc"], B["mybir"], B["TileContext"]
    ds, make_identity = B["ds"], B["make_identity"]
    dt = mybir.dt
    AX = mybir.AxisListType.X
    OP = mybir.AluOpType
    ACT = mybir.ActivationFunctionType
    D0P = D0C + 1
    SP0B = 8 * D0P
    SP0 = NB * SP0B

    lay, total_words = _layout(D0C)

    nc = bacc.Bacc("TRN2", target_bir_lowering=False)
    blob_d = nc.dram_tensor("blob", [1, total_words], dt.float32,
                            kind="ExternalInput")
    x0_d = nc.dram_tensor("x0", [ROWS0, IN_CH], dt.float32, kind="Internal")
    pidx0_d = nc.dram_tensor("pidx0", [16, SP0], dt.int16, kind="Internal")
    invdeg_d = nc.dram_tensor("invdeg", [R, 1], dt.float32, kind="Internal")
    cnt_d = nc.dram_tensor("cnt", [R, 1], dt.float32, kind="Internal")
    padmask_d = nc.dram_tensor("padmask", [128, NB], dt.float32,
                               kind="Internal")
    selfidx_d = nc.dram_tensor("selfidx", [16, NB * 8], dt.float32,
                               kind="Internal")
    qc_d = nc.dram_tensor("qc", [9, CCOL], dt.float32, kind="Internal")
    wxm_d = nc.dram_tensor("wxm", [128, HID], dt.float32, kind="Internal")
    wr1_d = nc.dram_tensor("wr1", [128, 4, HID], dt.float32, kind="Internal")
    wl1_d = nc.dram_tensor("wl1", [128, 4, HID], dt.float32, kind="Internal")
    wr2_d = nc.dram_tensor("wr2", [128, 4, HID], dt.float32, kind="Internal")
    wl2_d = nc.dram_tensor("wl2", [128, 4, HID], dt.float32, kind="Internal")
    vecs_d = nc.dram_tensor("vecs", [18, HID], dt.float32, kind="Internal")
    sc_d = nc.dram_tensor("sc", [128, 16], dt.float32, kind="Internal")
    _scatter_dst = {
        "x0": x0_d[:, :], "pidx0": pidx0_d[:, :], "invdeg": invdeg_d[:, :],
        "cnt": cnt_d[:, :], "padmask": padmask_d[:, :],
        "selfidx": selfidx_d[:, :], "qc": qc_d[:, :], "wxm": wxm_d[:, :],
        "wr1": wr1_d[:, :, :].rearrange("p a c -> p (a c)"),
        "wl1": wl1_d[:, :, :].rearrange("p a c -> p (a c)"),
        "wr2": wr2_d[:, :, :].rearrange("p a c -> p (a c)"),
        "wl2": wl2_d[:, :, :].rearrange("p a c -> p (a c)"),
        "vecs": vecs_d[:, :], "sc": sc_d[:, :],
    }

    xs_d = nc.dram_tensor("xs", [128, 12], dt.float32, kind="ExternalOutput")

    h_d = nc.dram_tensor("fh", [ROWS, 576], dt.float32, kind="Internal")
    a_d = nc.dram_tensor("fa", [ROWS, 64], dt.float32, kind="Internal")
    zb_d = nc.dram_tensor("zb", [R, 1], dt.float32, kind="Internal")
    z_d = nc.dram_tensor("z", [R, 1], dt.float32, kind="Internal")
    zm_d = nc.dram_tensor("zm", [1, R], dt.float32, kind="Internal")
    xn_d = nc.dram_tensor("xn", [R, HID], dt.float32, kind="Internal")
    xw_d = nc.dram_tensor("xw", [ROWS, HID], dt.float32, kind="Internal")
    knn_d = nc.dram_tensor("knn", [R, 16], dt.uint16, kind="Internal")
    msk_d = nc.dram_tensor("msk", [1, R], dt.float32, kind="Internal")
    pad_d = nc.dram_tensor("padc", [1, CCOL - R], dt.float32, kind="Internal")

    with TileContext(nc) as tc:
        with tc.tile_pool(name="const", bufs=1) as cpool:
            # scatter the blob to the per-tensor internal DRAM buffers
            for nm, (off, words, shape, kind) in lay.items():
                src = blob_d[0:1, off:off + words]
                if kind == "i2":
                    src = src.bitcast(dt.int16)
                src = src.rearrange("one (r c) -> r (c one)", r=shape[0])
                nc.sync.dma_start(_scatter_dst[nm], src)

            ident = cpool.tile([128, 128], dt.float32)
            make_identity(nc, ident[:])
            ones_r = cpool.tile([1, 128], dt.float32)
            nc.vector.memset(ones_r[:], 1.0)
            idbig = cpool.tile([128, 128], dt.float32)
            nc.vector.tensor_scalar_mul(idbig[:], ident[:], 1e30)
            padmask_sb = cpool.tile([128, NB], dt.float32)
            nc.sync.dma_start(padmask_sb[:], padmask_d[:, :])
            sc_sb = cpool.tile([128, 16], dt.float32)
            nc.sync.dma_start(sc_sb[:], sc_d[:, :])
            xs_sb = cpool.tile([128, 12], dt.float32)
            runmax = cpool.tile([128, HID], dt.float32)
            nc.vector.memset(runmax[:], -1e30)
            zm_sb = cpool.tile([128, NB], dt.float32)
            masks = [cpool.tile([128, NB], dt.float32, name="mask%d" % i)
                     for i in range(L)]
            tlos = [cpool.tile([1, 1], dt.float32, name="tlo%d" % i)
                    for i in range(L)]
            tlo128s = [cpool.tile([128, 1], dt.float32, name="tlo128%d" % i)
                       for i in range(L)]
            # bisection state
            lohi = cpool.tile([1, 2], dt.float32)
            tcur = cpool.tile([1, 1], dt.float32)
            cnts = cpool.tile([1, 1], dt.float32)
            isgt = cpool.tile([1, 1], dt.float32)
            d1 = cpool.tile([1, 1], dt.float32)
            cntp = cpool.tile([128, 1], dt.float32)
            mn = cpool.tile([128, 1], dt.float32)
            mx = cpool.tile([128, 1], dt.float32)
            t1r = cpool.tile([1, 128], dt.float32)
            ones_c = cpool.tile([128, 1], dt.float32)
            nc.vector.memset(ones_c[:], 1.0)
            # sentinel rows
            srow = cpool.tile([1, 576], dt.float32)
            nc.vector.memset(srow[:], 0.0)
            nc.vector.memset(srow[:, 512:513], -1e30)
            nc.sync.dma_start(h_d[SENT:SENT + 1, :], srow[:])
            nc.sync.dma_start(a_d[SENT:SENT + 1, :], srow[:, 0:64])

            def load_vecs(pool, li):
                t = {}
                for j, nm in enumerate(
                        ["br", "wq", "aw2", "l1w", "l2w", "l3w"]):
                    v = pool.tile([128, HID], dt.float32, tag="v_" + nm)
                    nc.sync.dma_start(
                        v[:], vecs_d[li * 6 + j:li * 6 + j + 1, :]
                        .to_broadcast([128, HID]))
                    t[nm] = v
                t["qb"] = sc_sb[:, li * 3 + 0:li * 3 + 1]
                t["l1b"] = sc_sb[:, li * 3 + 1:li * 3 + 2]
                t["l3b"] = sc_sb[:, li * 3 + 2:li * 3 + 3]
                return t

            def pool_fit_phases(wpool, pspool, idx_sb, idxoff, DP, V, li,
                                cnt_imm):
                """ASAPool + LEConv fitness over DP slots (incl self).

                idx_sb: [128, *] int16 gather tile; per-block window at
                idxoff(i) with 8*DP cols. cnt_imm: None -> load cnt_d.
                """
                SPB = 8 * DP

                def pool_body(i):
                    g = wpool.tile([128, DP, 576], dt.float32, tag="g")
                    nc.gpsimd.dma_gather(
                        out_ap=g[:], in_ap=h_d[:, :],
                        idxs_ap=idx_sb[:, idxoff(i)],
                        num_idxs=128 * DP, num_idxs_reg=128 * DP,
                        elem_size=576, single_packet=False)
                    xq = wpool.tile([128, DP // 2, HID], dt.float32, tag="xq")
                    _tree_max(nc,
                              lambda lo, c: g[:, lo:lo + c, 0:HID],
                              lambda lo, c: xq[:, lo:lo + c, :], DP)
                    tmp = wpool.tile([128, HID], dt.float32, tag="tmp")
                    nc.vector.tensor_mul(tmp[:], xq[:, 0, :], V["wq"][:])
                    qs = wpool.tile([128, 1], dt.float32, tag="qs")
                    nc.vector.tensor_reduce(qs[:], tmp[:], axis=AX, op=OP.add)
                    nc.vector.tensor_add(qs[:], qs[:], V["qb"])
                    sc = wpool.tile([128, DP], dt.float32, tag="sc")
                    jsv = g[:, :, 512:513].squeeze(2)
                    nc.vector.tensor_scalar_add(sc[:], jsv, qs[:])
                    sc2 = wpool.tile([128, DP], dt.float32, tag="sc2")
                    nc.vector.tensor_scalar_mul(sc2[:], sc[:], 0.2)
                    nc.vector.tensor_max(sc[:], sc[:], sc2[:])
                    m = wpool.tile([128, 1], dt.float32, tag="m")
                    nc.vector.tensor_reduce(m[:], sc[:], axis=AX, op=OP.max)
                    nc.vector.tensor_scalar(sc[:], sc[:], m[:], None,
                                            op0=OP.subtract)
                    nc.scalar.activation(sc[:], sc[:], ACT.Exp)
                    ssum = wpool.tile([128, 1], dt.float32, tag="ssum")
                    nc.vector.tensor_reduce(ssum[:], sc[:], axis=AX, op=OP.add)
                    rec = wpool.tile([128, 1], dt.float32, tag="rec")
                    nc.vector.reciprocal(rec[:], ssum[:])
                    nc.vector.tensor_scalar_mul(sc[:], sc[:], rec[:])
                    gh = g[:, :, 0:HID]
                    nc.vector.tensor_mul(
                        gh, gh, sc[:].unsqueeze(2).to_broadcast(
                            [128, DP, HID]))
                    _tree_sum(nc, lambda lo, c: g[:, lo:lo + c, 0:HID], DP)
                    xn = g[:, 0, 0:HID]
                    nc.sync.dma_start(xn_d[ds(i * 128, 128), :], xn)
                    nc.vector.tensor_mul(tmp[:], xn, V["l1w"][:])
                    av = wpool.tile([128, 1], dt.float32, tag="av")
                    nc.vector.tensor_reduce(av[:], tmp[:], axis=AX, op=OP.add)
                    nc.sync.dma_start(a_d[ds(i * 128, 128), 0:1], av[:])
                    nc.vector.tensor_mul(tmp[:], xn, V["l2w"][:])
                    bv = wpool.tile([128, 1], dt.float32, tag="bv")
                    nc.vector.tensor_reduce(bv[:], tmp[:], axis=AX, op=OP.add)
                    nc.vector.tensor_mul(tmp[:], xn, V["l3w"][:])
                    cv = wpool.tile([128, 1], dt.float32, tag="cv")
                    nc.vector.tensor_reduce(cv[:], tmp[:], axis=AX, op=OP.add)
                    zb = wpool.tile([128, 1], dt.float32, tag="zb")
                    lb1 = wpool.tile([128, 1], dt.float32, tag="lb1")
                    if cnt_imm is None:
                        ct = wpool.tile([128, 1], dt.float32, tag="ct")
                        nc.sync.dma_start(ct[:], cnt_d[ds(i * 128, 128), :])
                        nc.vector.tensor_mul(zb[:], ct[:], bv[:])
                        nc.vector.tensor_mul(lb1[:], ct[:], V["l1b"])
                    else:
                        nc.vector.tensor_scalar_mul(zb[:], bv[:], cnt_imm)
                        nc.vector.tensor_scalar_mul(lb1[:], V["l1b"], cnt_imm)
                    nc.vector.tensor_sub(zb[:], cv[:], zb[:])
                    nc.vector.tensor_add(zb[:], zb[:], V["l3b"])
                    nc.vector.tensor_add(zb[:], zb[:], lb1[:])
                    nc.sync.dma_start(zb_d[ds(i * 128, 128), :], zb[:])

                with tc.For_i(0, NB) as i:
                    pool_body(i)

                def fit_body(i):
                    ga = wpool.tile([128, DP, 64], dt.float32, tag="ga")
                    nc.gpsimd.dma_gather(
                        out_ap=ga[:], in_ap=a_d[:, :],
                        idxs_ap=idx_sb[:, idxoff(i)],
                        num_idxs=128 * DP, num_idxs_reg=128 * DP,
                        elem_size=64, single_packet=False)
                    zs = wpool.tile([128, 1], dt.float32, tag="zs")
                    nc.vector.tensor_reduce(zs[:], ga[:, :, 0:1].squeeze(2),
                                            axis=AX, op=OP.add)
                    zbl = wpool.tile([128, 1], dt.float32, tag="zbl")
                    nc.sync.dma_start(zbl[:], zb_d[ds(i * 128, 128), :])
                    nc.vector.tensor_add(zs[:], zs[:], zbl[:])
                    nc.sync.dma_start(z_d[ds(i * 128, 128), :], zs[:])

                with tc.For_i(0, NB) as i:
                    fit_body(i)

            def thresh_phase(li, prevmask):
                """zm = z + prevmask; bisect to top-KTGT threshold."""
                with tc.tile_pool(name="tps%d" % li, bufs=1,
                                  space="PSUM") as pst:
                    nc.sync.dma_start(
                        zm_sb[:],
                        z_d[:, :].rearrange("(b p) one -> p (b one)", p=128))
                    nc.vector.tensor_add(zm_sb[:], zm_sb[:], prevmask[:])
                    nc.sync.dma_start(
                        zm_d[0:1, :].rearrange("one (b p) -> p (b one)",
                                               p=128), zm_sb[:])
                    # active min/max for lo/hi
                    neg = cpool.tile([128, NB], dt.float32, tag="neg%d" % li)
                    nc.vector.tensor_scalar_mul(neg[:], prevmask[:], -1.0)
                    nc.vector.tensor_add(neg[:], neg[:], zm_sb[:])
                    nc.vector.tensor_reduce(mn[:], neg[:], axis=AX, op=OP.min)
                    nc.vector.tensor_reduce(mx[:], zm_sb[:], axis=AX,
                                            op=OP.max)
                    tpr = pst.tile([1, 128], dt.float32, tag="tpr")
                    nc.tensor.transpose(tpr[:], mn[:], ident[:])
                    nc.vector.tensor_copy(t1r[:], tpr[:])
                    nc.vector.tensor_reduce(lohi[:, 0:1], t1r[:], axis=AX,
                                            op=OP.min)
                    nc.tensor.transpose(tpr[:], mx[:], ident[:])
                    nc.vector.tensor_copy(t1r[:], tpr[:])
                    nc.vector.tensor_reduce(lohi[:, 1:2], t1r[:], axis=AX,
                                            op=OP.max)
                    ktgt = KTGT[li]
                    tbp = pst.tile([128, 1], dt.float32, tag="tbp")
                    cnt1 = pst.tile([1, 1], dt.float32, tag="cnt1")
                    with tc.For_i(0, BIS) as it:
                        nc.vector.tensor_add(tcur[:], lohi[:, 0:1],
                                             lohi[:, 1:2])
                        nc.vector.tensor_scalar_mul(tcur[:], tcur[:], 0.5)
                        nc.tensor.matmul(tbp[:], ones_r[:], tcur[:],
                                         start=True, stop=True)
                        tbs = cpool.tile([128, 1], dt.float32, tag="tbs")
                        nc.vector.tensor_copy(tbs[:], tbp[:])
                        cmp = cpool.tile([128, NB], dt.float32, tag="cmp")
                        nc.vector.tensor_scalar(cmp[:], zm_sb[:], tbs[:],
                                                None, op0=OP.is_gt)
                        nc.vector.tensor_reduce(cntp[:], cmp[:], axis=AX,
                                                op=OP.add)
                        nc.tensor.matmul(cnt1[:], cntp[:], ones_c[:],
                                         start=True, stop=True)
                        nc.vector.tensor_copy(cnts[:], cnt1[:])
                        nc.vector.tensor_scalar(isgt[:], cnts[:], ktgt - 0.5,
                                                None, op0=OP.is_gt)
                        nc.vector.tensor_sub(d1[:], tcur[:], lohi[:, 0:1])
                        nc.vector.tensor_mul(d1[:], d1[:], isgt[:])
                        nc.vector.tensor_add(lohi[:, 0:1], lohi[:, 0:1],
                                             d1[:])
                        nc.vector.tensor_sub(d1[:], lohi[:, 1:2], tcur[:])
                        nc.vector.tensor_mul(d1[:], d1[:], isgt[:])
                        nc.vector.tensor_add(lohi[:, 1:2], tcur[:], d1[:])
                    nc.vector.tensor_copy(tlos[li][:], lohi[:, 0:1])
                    nc.tensor.matmul(tbp[:], ones_r[:], tlos[li][:],
                                     start=True, stop=True)
                    nc.vector.tensor_copy(tlo128s[li][:], tbp[:])
                    # maskadd = (zm > tlo ? 0 : -3e30)
                    nc.vector.tensor_scalar(masks[li][:], zm_sb[:],
                                            tlo128s[li][:], None,
                                            op0=OP.is_gt)
                    nc.vector.tensor_scalar_add(masks[li][:], masks[li][:],
                                                -1.0)
                    nc.vector.tensor_scalar_mul(masks[li][:], masks[li][:],
                                                3e30)

            def xw_xs_phase(li, write_xw):
                """xw = xn*sigmoid(zm) (-> xw_d), masked running max -> xs."""
                with (
                    tc.tile_pool(name="wp_w%d" % li, bufs=2) as wp,
                    tc.tile_pool(name="psw%d" % li, bufs=1,
                                 space="PSUM") as psw,
                ):
                    def w_body(i):
                        fv = wp.tile([128, 1], dt.float32, tag="fv")
                        nc.scalar.activation(fv[:], zm_sb[:, ds(i, 1)],
                                             ACT.Sigmoid)
                        xnb = wp.tile([128, HID], dt.float32, tag="xnb")
                        nc.sync.dma_start(xnb[:], xn_d[ds(i * 128, 128), :])
                        xw = wp.tile([128, HID], dt.float32, tag="xw")
                        nc.vector.tensor_scalar_mul(xw[:], xnb[:], fv[:])
                        if write_xw:
                            nc.sync.dma_start(xw_d[ds(i * 128, 128), :],
                                              xw[:])
                        nc.vector.tensor_scalar_add(
                            xw[:], xw[:], masks[li][:, ds(i, 1)])
                        nc.vector.tensor_max(runmax[:], runmax[:], xw[:])

                    with tc.For_i(0, NB) as i:
                        w_body(i)
                    # reduce runmax across partitions into xs_sb cols
                    for c in range(4):
                        tp = psw.tile([128, 128], dt.float32,
                                      tag="tp%d" % (c % 2))
                        nc.tensor.transpose(tp[:],
                                            runmax[:, c * 128:(c + 1) * 128],
                                            ident[:])
                        nc.vector.tensor_reduce(
                            xs_sb[:, li * 4 + c:li * 4 + c + 1], tp[:],
                            axis=AX, op=OP.max)
                    nc.vector.memset(runmax[:], -1e30)

            def knn_phase(li):
                """Masked dense kNN sweep -> knn_d (16 candidates/node)."""
                with (
                    tc.tile_pool(name="wp_k%d" % li, bufs=2) as wp,
                    tc.tile_pool(name="psk%d" % li, bufs=2,
                                 space="PSUM") as psk,
                ):
                    cand = wp.tile([5, CCOL], dt.float32, tag="cand", bufs=1)
                    nc.sync.dma_start(cand[0:4, :], qc_d[5:9, :])
                    row = wp.tile([128, CCOL], dt.float32, tag="row", bufs=1)
                    # mask row: reuse row[0:1] as scratch, then bounce via
                    # DRAM for the partition shift 0 -> 4
                    zmr = row[0:1, 0:R]
                    nc.sync.dma_start(zmr, zm_d[:, :])
                    nc.vector.tensor_scalar(zmr, zmr,
                                            tlos[li][:], None, op0=OP.is_gt)
                    nc.vector.tensor_scalar_add(zmr, zmr, -1.0)
                    nc.vector.tensor_scalar_mul(zmr, zmr, 3e30)
                    nc.sync.dma_start(msk_d[:, :], zmr)
                    nc.sync.dma_start(cand[4:5, 0:R], msk_d[:, :])
                    if CCOL > R:
                        padc = wp.tile([1, CCOL - R], dt.float32, tag="padc")
                        nc.vector.memset(padc[:], -3e30)
                        nc.sync.dma_start(pad_d[:, :], padc[:])
                        nc.sync.dma_start(cand[4:5, R:CCOL], pad_d[:, :])

                    HCOL = CCOL // 2

                    def k_body(i):
                        qsb = wp.tile([5, 128], dt.float32, tag="qsb")
                        nc.sync.dma_start(qsb[:], qc_d[0:5, ds(i * 128, 128)])
                        for ch in range(CH):
                            dps = psk.tile([128, 512], dt.float32,
                                           tag="d%d" % (ch % 2))
                            nc.tensor.matmul(dps[:], qsb[:],
                                             cand[:, ch * 512:(ch + 1) * 512],
                                             start=True, stop=True)
                            nc.scalar.activation(
                                row[:, ch * 512:(ch + 1) * 512], dps[:],
                                ACT.Copy)
                        # self-exclusion on the diagonal block
                        nc.vector.tensor_sub(row[:, ds(i * 128, 128)],
                                             row[:, ds(i * 128, 128)],
                                             idbig[:])
                        # per-half top-8 (max8 input cap is 16384)
                        vab = wp.tile([128, 16], dt.float32, tag="vab")
                        iab = wp.tile([128, 16], dt.float32, tag="iab")
                        vA = wp.tile([128, 8], dt.float32, tag="vA")
                        iA = wp.tile([128, 8], dt.uint32, tag="iA")
                        nc.vector.max(out=vA[:], in_=row[:, 0:HCOL])
                        nc.vector.max_index(iA[:], vA[:], row[:, 0:HCOL])
                        nc.vector.tensor_copy(vab[:, 0:8], vA[:])
                        nc.vector.tensor_copy(iab[:, 0:8], iA[:])
                        vB = wp.tile([128, 8], dt.float32, tag="vB")
                        iB = wp.tile([128, 8], dt.uint32, tag="iB")
                        nc.vector.max(out=vB[:], in_=row[:, HCOL:CCOL])
                        nc.vector.max_index(iB[:], vB[:], row[:, HCOL:CCOL])
                        nc.vector.tensor_copy(vab[:, 8:16], vB[:])
                        nc.vector.tensor_copy(iab[:, 8:16], iB[:])
                        nc.vector.tensor_scalar_add(iab[:, 8:16],
                                                    iab[:, 8:16],
                                                    float(HCOL))
                        # iab1 = idx + 1 (0 must not survive the eq-mask max)
                        nc.vector.tensor_scalar_add(iab[:], iab[:], 1.0)
                        # merge the two sorted top-8 lists -> top-10 distinct
                        outf = wp.tile([128, 16], dt.float32, tag="outf")
                        mm = wp.tile([128, 1], dt.float32, tag="mm")
                        eq = wp.tile([128, 16], dt.float32, tag="eq")
                        tmq = wp.tile([128, 16], dt.float32, tag="tmq")
                        for s in range(10):
                            nc.vector.tensor_reduce(mm[:], vab[:], axis=AX,
                                                    op=OP.max)
                            nc.vector.tensor_scalar(eq[:], vab[:], mm[:],
                                                    None, op0=OP.is_equal)
                            nc.vector.tensor_mul(tmq[:], eq[:], iab[:])
                            nc.vector.tensor_reduce(outf[:, s:s + 1], tmq[:],
                                                    axis=AX, op=OP.max)
                            nc.vector.tensor_scalar_mul(tmq[:], eq[:], -1e31)
                            nc.vector.tensor_add(vab[:], vab[:], tmq[:])
                        nc.vector.tensor_scalar_add(outf[:, 0:10],
                                                    outf[:, 0:10], -1.0)
                        i16 = wp.tile([128, 16], dt.uint16, tag="i16")
                        nc.vector.tensor_copy(i16[:, 0:10], outf[:, 0:10])
                        nc.sync.dma_start(knn_d[ds(i * 128, 128), 0:10],
                                          i16[:, 0:10])

                    with tc.For_i(0, NB) as i:
                        k_body(i)

            def idx_build_phase(pool, li, D):
                """Build [128, NB*(D+1)*8] int16 gather tile from knn_d."""
                W = (D + 1) * 8
                idxg = pool.tile([128, NB * W], dt.int16, tag="idxg")
                with (
                    tc.tile_pool(name="ib%d" % li, bufs=2) as ib,
                    tc.tile_pool(name="psib%d" % li, bufs=2,
                                 space="PSUM") as psib,
                ):
                    stage = ib.tile([16, NB, D + 1, 8], dt.float32,
                                    tag="stage", bufs=1)
                    nc.sync.dma_start(
                        stage[:, :, D, :],
                        selfidx_d[:, :].rearrange("p (b c) -> p b c", b=NB))

                    def ib_body(i):
                        kb = ib.tile([128, 16], dt.uint16, tag="kb")
                        nc.sync.dma_start(kb[:], knn_d[ds(i * 128, 128), :])
                        kf = ib.tile([128, 16], dt.float32, tag="kf")
                        nc.vector.tensor_copy(kf[:], kb[:])
                        t1ps = psib.tile([16, 128], dt.float32, tag="t1ps")
                        nc.tensor.transpose(t1ps[:], kf[:], ident[:])
                        t1t = ib.tile([16, 128], dt.float32, tag="t1t")
                        nc.vector.tensor_copy(t1t[:], t1ps[:])
                        for c8 in range(8):
                            t2ps = psib.tile([16, 16], dt.float32,
                                             tag="t2_%d" % (c8 % 2))
                            nc.tensor.transpose(t2ps[:],
                                                t1t[:, c8 * 16:(c8 + 1) * 16],
                                                ident[0:16, 0:16])
                            nc.vector.tensor_copy(
                                stage[:, ds(i, 1), 0:D, c8].squeeze(1),
                                t2ps[:, 0:D])

                    with tc.For_i(0, NB) as i:
                        ib_body(i)
                    idx16 = ib.tile([16, NB * W], dt.int16, tag="idx16",
                                    bufs=1)
                    nc.vector.tensor_copy(
                        idx16[:],
                        stage[:].rearrange("p a b c -> p (a b c)"))
                    for g in range(8):
                        nc.sync.dma_start(idxg[g * 16:(g + 1) * 16, :],
                                          idx16[:])
                return idxg

            def conv_knn_phase(wpool, pspool, idxg, D, V, wr_t, wl_t):
                """GraphConv over the kNN graph (D neighbors + self slot)."""
                DP = D + 1
                WB = DP * 8

                def c_body(i):
                    g = wpool.tile([128, DP, HID], dt.float32, tag="cg")
                    nc.gpsimd.dma_gather(
                        out_ap=g[:], in_ap=xw_d[:, :],
                        idxs_ap=idxg[:, ds(i * WB, WB)],
                        num_idxs=128 * DP, num_idxs_reg=128 * DP,
                        elem_size=HID, single_packet=False)
                    _tree_sum(nc, lambda lo, c: g[:, lo:lo + c, :], D)
                    mean = wpool.tile([128, HID], dt.float32, tag="mean")
                    nc.vector.tensor_scalar_mul(mean[:], g[:, 0, :], 1.0 / D)
                    hps = pspool.tile([128, HID], dt.float32, tag="hps")
                    xt = wpool.tile([128, 4, 128], dt.float32, tag="xt")
                    mt = wpool.tile([128, 4, 128], dt.float32, tag="mt")
                    for c in range(4):
                        tp = pspool.tile([128, 128], dt.float32,
                                         tag="tp%d" % (c % 2))
                        nc.tensor.transpose(tp[:],
                                            g[:, D, c * 128:(c + 1) * 128],
                                            ident[:])
                        nc.vector.tensor_copy(xt[:, c, :], tp[:])
                        tp2 = pspool.tile([128, 128], dt.float32,
                                          tag="tq%d" % (c % 2))
                        nc.tensor.transpose(tp2[:],
                                            mean[:, c * 128:(c + 1) * 128],
                                            ident[:])
                        nc.vector.tensor_copy(mt[:, c, :], tp2[:])
                    for c in range(4):
                        nc.tensor.matmul(hps[:], xt[:, c, :], wl_t[:, c, :],
                                         start=(c == 0), stop=False)
                    for c in range(4):
                        nc.tensor.matmul(hps[:], mt[:, c, :], wr_t[:, c, :],
                                         start=False, stop=(c == 3))
                    hsb = wpool.tile([128, 576], dt.float32, tag="hsb")
                    nc.vector.tensor_add(hsb[:, 0:HID], hps[:], V["br"][:])
                    nc.vector.tensor_scalar_max(hsb[:, 0:HID], hsb[:, 0:HID],
                                                0.0)
                    tmp = wpool.tile([128, HID], dt.float32, tag="ctmp")
                    nc.vector.tensor_mul(tmp[:], hsb[:, 0:HID], V["aw2"][:])
                    nc.vector.tensor_reduce(hsb[:, 512:513], tmp[:], axis=AX,
                                            op=OP.add)
                    nc.sync.dma_start(h_d[ds(i * 128, 128), 0:513],
                                      hsb[:, 0:513])

                with tc.For_i(0, NB) as i:
                    c_body(i)

            # ================= layer 0 =================
            with tc.tile_pool(name="seg0", bufs=1) as seg0:
                pidx0_sb = seg0.tile([128, SP0], dt.int16, tag="pidx0")
                for g in range(8):
                    nc.sync.dma_start(pidx0_sb[g * 16:(g + 1) * 16, :],
                                      pidx0_d[:, :])
                V0 = load_vecs(seg0, 0)
                wxm_sb = seg0.tile([128, HID], dt.float32, tag="wxm")
                nc.sync.dma_start(wxm_sb[:], wxm_d[:, :])

                with (
                    tc.tile_pool(name="l0c", bufs=2) as wp0,
                    tc.tile_pool(name="ps0", bufs=2, space="PSUM") as ps0,
                ):
                    def conv0_body(i):
                        g = wp0.tile([128, D0P, IN_CH], dt.float32, tag="g0")
                        nc.gpsimd.dma_gather(
                            out_ap=g[:], in_ap=x0_d[:, :],
                            idxs_ap=pidx0_sb[:, ds(i * SP0B, SP0B)],
                            num_idxs=128 * D0P, num_idxs_reg=128 * D0P,
                            elem_size=IN_CH, single_packet=False)
                        _tree_sum(nc, lambda lo, c: g[:, lo:lo + c, :], D0P)
                        xm = wp0.tile([128, 128], dt.float32, tag="xm")
                        nc.sync.dma_start(xm[:, 0:IN_CH],
                                          x0_d[ds(i * 128, 128), :])
                        # neighbors-only sum = sum(all slots) - self
                        nc.vector.tensor_sub(g[:, 0, :], g[:, 0, :],
                                             xm[:, 0:IN_CH])
                        iv = wp0.tile([128, 1], dt.float32, tag="iv")
                        nc.sync.dma_start(iv[:],
                                          invdeg_d[ds(i * 128, 128), :])
                        nc.vector.tensor_scalar_mul(xm[:, IN_CH:2 * IN_CH],
                                                    g[:, 0, :], iv[:])
                        tp = ps0.tile([128, 128], dt.float32, tag="tp0")
                        nc.tensor.transpose(tp[:], xm[:], ident[:])
                        lhsT = wp0.tile([128, 128], dt.float32, tag="lhsT")
                        nc.vector.tensor_copy(lhsT[:], tp[:])
                        hps = ps0.tile([128, HID], dt.float32, tag="hps0")
                        nc.tensor.matmul(hps[:], lhsT[:], wxm_sb[:],
                                         start=True, stop=True)
                        hsb = wp0.tile([128, 576], dt.float32, tag="hsb0")
                        nc.vector.tensor_add(hsb[:, 0:HID], hps[:],
                                             V0["br"][:])
                        nc.vector.tensor_scalar_max(hsb[:, 0:HID],
                                                    hsb[:, 0:HID], 0.0)
                        tmp = wp0.tile([128, HID], dt.float32, tag="tmp0")
                        nc.vector.tensor_mul(tmp[:], hsb[:, 0:HID],
                                             V0["aw2"][:])
                        nc.vector.tensor_reduce(hsb[:, 512:513], tmp[:],
                                                axis=AX, op=OP.add)
                        nc.sync.dma_start(h_d[ds(i * 128, 128), 0:513],
                                          hsb[:, 0:513])

                    with tc.For_i(0, NB) as i:
                        conv0_body(i)
                with (
                    tc.tile_pool(name="l0p", bufs=2) as wp0p,
                    tc.tile_pool(name="ps0p", bufs=2, space="PSUM") as ps0p,
                ):
                    pool_fit_phases(wp0p, ps0p, pidx0_sb,
                                    lambda i: ds(i * SP0B, SP0B), D0P, V0, 0,
                                    None)
            thresh_phase(0, padmask_sb)
            xw_xs_phase(0, True)
            knn_phase(0)

            # ================= layer 1 =================
            with tc.tile_pool(name="seg1", bufs=1) as seg1:
                idxg1 = idx_build_phase(seg1, 1, K1)
                with (
                    tc.tile_pool(name="l1", bufs=2) as wp1,
                    tc.tile_pool(name="ps1", bufs=1, space="PSUM") as ps1,
                ):
                    V1 = load_vecs(wp1, 1)
                    wr1_sb = wp1.tile([128, 4, HID], dt.float32, tag="wr",
                                      bufs=1)
                    nc.sync.dma_start(wr1_sb[:], wr1_d[:, :, :])
                    wl1_sb = wp1.tile([128, 4, HID], dt.float32, tag="wl",
                                      bufs=1)
                    nc.sync.dma_start(wl1_sb[:], wl1_d[:, :, :])
                    conv_knn_phase(wp1, ps1, idxg1, K1, V1, wr1_sb, wl1_sb)
                    pool_fit_phases(wp1, ps1, idxg1,
                                    lambda i: ds(i * (K1 + 1) * 8,
                                                 (K1 + 1) * 8),
                                    K1 + 1, V1, 1, float(K1 + 1))
            thresh_phase(1, masks[0])
            xw_xs_phase(1, True)
            knn_phase(1)

            # ================= layer 2 =================
            with tc.tile_pool(name="seg2", bufs=1) as seg2:
                idxg2 = idx_build_phase(seg2, 2, K2)
                with (
                    tc.tile_pool(name="l2", bufs=2) as wp2,
                    tc.tile_pool(name="ps2", bufs=1, space="PSUM") as ps2,
                ):
                    V2 = load_vecs(wp2, 2)
                    wr2_sb = wp2.tile([128, 4, HID], dt.float32, tag="wr",
                                      bufs=1)
                    nc.sync.dma_start(wr2_sb[:], wr2_d[:, :, :])
                    wl2_sb = wp2.tile([128, 4, HID], dt.float32, tag="wl",
                                      bufs=1)
                    nc.sync.dma_start(wl2_sb[:], wl2_d[:, :, :])
                    conv_knn_phase(wp2, ps2, idxg2, K2, V2, wr2_sb, wl2_sb)
                    pool_fit_phases(wp2, ps2, idxg2,
                                    lambda i: ds(i * (K2 + 1) * 8,
                                                 (K2 + 1) * 8),
                                    K2 + 1, V2, 2, float(K2 + 1))
            thresh_phase(2, masks[1])
            xw_xs_phase(2, False)

            nc.sync.dma_start(xs_d[:, :], xs_sb[:])
    nc.compile()
    return nc


# ----------------------------------------------------------------------------
# build/compile management (import-time warm-up)
# ----------------------------------------------------------------------------

_RUNNERS = {}
_BUILD_LOCK = threading.Lock()
_BUILD_THREADS = []
_READY = {"F_%d" % D0C_DEFAULT: threading.Event()}
_CACHE_DIR = "/tmp/asap_gnn_v3_cache"


def _cache_path(D0C):
    import hashlib
    import inspect
    try:
        srcs = inspect.getsource(_build_F)
    except Exception:
        srcs = "nosrc"
    key = "%s|%d|%d|%d|%d|%d|%d|%s" % (
        srcs, D0C, NB, CCOL, BIS, K1, K2, KTGT)
    h = hashlib.sha1(key.encode()).hexdigest()[:16]
    return "%s/F_%d_%s.pkl" % (_CACHE_DIR, D0C, h)


def _load_meta(D0C):
    import pickle
    try:
        with open(_cache_path(D0C), "rb") as f:
            return pickle.load(f)
    except Exception:
        return None


def _save_meta(D0C, meta):
    import os
    import pickle
    try:
        os.makedirs(_CACHE_DIR, exist_ok=True)
        d = {k: v for k, v in meta.items() if k != "ncobj"}
        tmp = _cache_path(D0C) + ".tmp.%d" % os.getpid()
        with open(tmp, "wb") as f:
            pickle.dump(d, f)
        os.replace(tmp, _cache_path(D0C))
    except Exception:
        pass


def _make_launcher(D0C):
    meta = _load_meta(D0C)
    if meta is None:
        meta = _meta_from_nc(_build_F(D0C))
        _save_meta(D0C, meta)
    return _Launcher(meta).warm()


def _get_runner(name, D0C):
    with _BUILD_LOCK:
        if name in _RUNNERS:
            return _RUNNERS[name]
    r = _make_launcher(D0C)
    with _BUILD_LOCK:
        _RUNNERS.setdefault(name, r)
    return _RUNNERS[name]


def _warm():
    try:
        jini = threading.Thread(target=lambda: _get_bass()["jax"].devices())
        jini.start()
        name = "F_%d" % D0C_DEFAULT
        l = _make_launcher(D0C_DEFAULT)
        with _BUILD_LOCK:
            _RUNNERS.setdefault(name, l)
        jini.join()
    except Exception:  # pragma: no cover - fallback path handles
        import traceback
        traceback.print_exc()
    finally:
        for ev in _READY.values():
            ev.set()


_BUILD_THREADS.append(threading.Thread(target=_warm, daemon=True))
_BUILD_THREADS[-1].start()


# ----------------------------------------------------------------------------
# numpy fallback (used only if the device path fails)
# ----------------------------------------------------------------------------

def _np_reference(x, pos, src, dst, W):
    f = _f32
    n = N0
    xs = []
    for i in range(L):
        wr, br, wl = W["wr"][i], W["br"][i], W["wl"][i]
        agg = np.zeros((n, x.shape[1]), f)
        np.add.at(agg, dst, x[src])
        deg = np.bincount(dst, minlength=n).astype(f)
        mean = agg / np.maximum(deg, 1)[:, None]
        h = np.maximum(mean @ wr + br + x @ wl, 0).astype(f)
        sl = np.arange(n)
        s_ = np.concatenate([src, sl])
        d_ = np.concatenate([dst, sl])
        xj = h[s_]
        xq = np.full((n, HID), -np.inf, f)
        np.maximum.at(xq, d_, xj)
        xq = (xq @ W["lw"][i] + W["lb"][i]).astype(f)
        aw, ab = W["aw"][i], W["ab"][i]
        score = (xq[d_] @ aw[:HID] + xj @ aw[HID:] + ab).astype(f)
        score = np.where(score > 0, score, f(0.2) * score).astype(f)
        smax = np.full(n, -np.inf, f)
        np.maximum.at(smax, d_, score)
        ex = np.exp(score - smax[d_])
        ssum = np.zeros(n, f)
        np.add.at(ssum, d_, ex)
        att = (ex / ssum[d_]).astype(f)
        xn = np.zeros((n, HID), f)
        np.add.at(xn, d_, xj * att[:, None])
        a = xn @ W["l1w"][i] + W["l1b"][i]
        b = xn @ W["l2w"][i]
        agg2 = np.zeros(n, f)
        np.add.at(agg2, d_, (a[s_] - b[d_]).astype(f))
        z = (agg2 + xn @ W["l3w"][i] + W["l3b"][i]).astype(f)
        k_keep = int(math.ceil(RATIO * n))
        fit64 = 1.0 / (1.0 + np.exp(-z.astype(np.float64)))
        perm = np.argpartition(-fit64, k_keep - 1)[:k_keep]
        fv = fit64[perm].astype(f)
        x = (xn[perm] * fv[:, None]).astype(f)
        xs.append(x.max(0))
        pos = pos[perm]
        n = k_keep
        if i < L - 1:
            k = 6 + 2 * i
            sq = np.sum(pos * pos, -1)
            dist = sq[:, None] + sq[None, :] - 2 * (pos @ pos.T)
            np.fill_diagonal(dist, np.inf)
            idx = np.argpartition(dist, k, 1)[:, :k]
            srt = np.take_along_axis(dist, idx, 1).argsort(1, kind="stable")
            idx = np.take_along_axis(idx, srt, 1)
            dst = np.repeat(np.arange(n), k)
            src = idx.reshape(-1)
    return xs


# ----------------------------------------------------------------------------
# main kernel
# ----------------------------------------------------------------------------

_EXEC_NS = []


def kernel(x, pos, edge_index, conv0_wr, conv0_br, conv0_wl, conv_wr, conv_br,
           conv_wl, pool_lin_w, pool_lin_b, pool_att_w, pool_att_b, le1_w,
           le1_b, le2_w, le3_w, le3_b, lin1_w, lin1_b, lin2_w, lin2_b):
    t_start = time.perf_counter()
    _EXEC_NS.clear()
    x = np.asarray(x, _f32)
    pos = np.asarray(pos, _f32)
    ei = np.asarray(edge_index).astype(np.int64)

    W = {
        "wr": [np.asarray(conv0_wr, _f32)] + [np.asarray(conv_wr[i], _f32)
                                              for i in range(L - 1)],
        "br": [np.asarray(conv0_br, _f32)] + [np.asarray(conv_br[i], _f32)
                                              for i in range(L - 1)],
        "wl": [np.asarray(conv0_wl, _f32)] + [np.asarray(conv_wl[i], _f32)
                                              for i in range(L - 1)],
        "lw": [np.asarray(pool_lin_w[i], _f32) for i in range(L)],
        "lb": [np.asarray(pool_lin_b[i], _f32) for i in range(L)],
        "aw": [np.asarray(pool_att_w[i], _f32) for i in range(L)],
        "ab": [float(pool_att_b[i]) for i in range(L)],
        "l1w": [np.asarray(le1_w[i], _f32) for i in range(L)],
        "l1b": [float(le1_b[i]) for i in range(L)],
        "l2w": [np.asarray(le2_w[i], _f32) for i in range(L)],
        "l3w": [np.asarray(le3_w[i], _f32) for i in range(L)],
        "l3b": [float(le3_b[i]) for i in range(L)],
    }
    try:
        xs = _device_forward(x, pos, ei, W)
    except Exception:
        import traceback
        traceback.print_exc()
        print("kernel: device path failed; numpy fallback")
        xs = _np_reference(x, pos, ei[0], ei[1], W)

    hcat = np.concatenate(xs)[None, :].astype(_f32)
    h1 = np.maximum(hcat @ np.asarray(lin1_w, _f32) +
                    np.asarray(lin1_b, _f32), 0)
    out = (h1 @ np.asarray(lin2_w, _f32) + np.asarray(lin2_b, _f32))
    dt_ns = int((time.perf_counter() - t_start) * 1e9)
    _EXEC_NS.append(("kernel", dt_ns))
    return out.astype(_f32)


def _device_forward(x, pos, ei, W):
    _T0 = time.perf_counter()
    src, dst = ei[0], ei[1]

    # ---------------- host prep (pure numpy, overlaps warm) ----------------
    deg0 = np.bincount(dst, minlength=R).astype(np.int64)
    D0C = max(int(deg0.max()), 1)
    name = "F_%d" % D0C

    x0 = np.zeros((ROWS0, IN_CH), _f32)
    x0[:N0] = x
    tblC, _ = _slot_table(src, dst, R, D0C, SENT)
    tblP = np.concatenate(
        [np.arange(R, dtype=np.int64)[:, None], tblC], 1)
    tblP[N0:, 0] = SENT
    pidx0 = _idx_to_i16_tile(_slotmajor_list(tblP))
    invdeg0 = (1.0 / np.maximum(deg0, 1.0)).astype(_f32)[:, None]
    cnt0 = (deg0 + 1).astype(_f32)[:, None]
    padmask = np.zeros((128, NB), _f32)
    for j in range(N0, R):
        padmask[j % 128, j // 128] = -3e30
    selfidx = np.zeros((16, NB * 8), _f32)
    ar = np.arange(R).reshape(NB, 8, 16)           # [b, c8, p16]
    selfidx[:, :] = ar.transpose(2, 0, 1).reshape(16, NB * 8)
    sq = np.sum(pos * pos, 1, dtype=_f32)
    qc = np.zeros((9, CCOL), _f32)
    qc[0, :N0] = 2.0 * pos[:, 0]
    qc[1, :N0] = 2.0 * pos[:, 1]
    qc[2, :N0] = -1.0
    qc[3, :N0] = -sq
    qc[4, :N0] = 1.0
    qc[5, :N0] = pos[:, 0]
    qc[6, :N0] = pos[:, 1]
    qc[7, :N0] = sq
    qc[7, N0:] = 1e30
    qc[8, :] = 1.0
    wxm = np.zeros((128, HID), _f32)
    wxm[0:IN_CH] = W["wl"][0]
    wxm[IN_CH:2 * IN_CH] = W["wr"][0]
    vecs = np.zeros((18, HID), _f32)
    sc = np.zeros((128, 16), _f32)
    for li in range(L):
        lw, lb = W["lw"][li], W["lb"][li]
        aw, ab = W["aw"][li], W["ab"][li]
        wq = (lw @ aw[:HID]).astype(_f32)
        qb = float(lb @ aw[:HID] + ab)
        vecs[li * 6 + 0] = W["br"][li] if li == 0 else W["br"][li]
        vecs[li * 6 + 1] = wq
        vecs[li * 6 + 2] = aw[HID:]
        vecs[li * 6 + 3] = W["l1w"][li]
        vecs[li * 6 + 4] = W["l2w"][li]
        vecs[li * 6 + 5] = W["l3w"][li]
        sc[:, li * 3 + 0] = qb
        sc[:, li * 3 + 1] = W["l1b"][li]
        sc[:, li * 3 + 2] = W["l3b"][li]
    wpack = {}
    for li in (1, 2):
        wpack["wr%d" % li] = np.ascontiguousarray(
            W["wr"][li].reshape(4, 128, HID).transpose(1, 0, 2))
        wpack["wl%d" % li] = np.ascontiguousarray(
            W["wl"][li].reshape(4, 128, HID).transpose(1, 0, 2))

    host = {"x0": x0, "pidx0": pidx0, "invdeg": invdeg0, "cnt": cnt0,
            "padmask": padmask, "selfidx": selfidx, "qc": qc, "wxm": wxm,
            "wr1": wpack["wr1"], "wl1": wpack["wl1"], "wr2": wpack["wr2"],
            "wl2": wpack["wl2"], "vecs": vecs, "sc": sc}
    lay, total_words = _layout(D0C)
    blob = np.empty((1, total_words), _f32)
    for nm, (off, words, shape, kind) in lay.items():
        a = host[nm]
        if kind == "i2":
            blob[0, off:off + words] = np.ascontiguousarray(
                a).reshape(-1).view(_f32)
        else:
            blob[0, off:off + words] = a.reshape(-1)
    _EXEC_NS.append(("prep", int((time.perf_counter() - _T0) * 1e9)))

    # ---------------- upload (overlaps AOT warm in the bg thread) ----------
    t0 = time.perf_counter()
    B = _get_bass()
    jax = B["jax"]
    dev = jax.devices()[0]
    inF = {"blob": jax.device_put(blob, dev)}
    inF["blob"].block_until_ready()
    _EXEC_NS.append(("puts", int((time.perf_counter() - t0) * 1e9)))

    # ---------------- wait program, launch, download -----------------------
    t0 = time.perf_counter()
    ev = _READY.get(name)
    if ev is not None:
        ev.wait()
    Frun = _RUNNERS.get(name) or _get_runner(name, D0C)
    _EXEC_NS.append(("warmjoin", int((time.perf_counter() - t0) * 1e9)))

    t0 = time.perf_counter()
    rF = Frun(inF)
    xs_t = np.asarray(rF["xs"])       # [128, 12]
    _EXEC_NS.append(("exec", int((time.perf_counter() - t0) * 1e9)))

    xs = []
    for li in range(L):
        v = np.empty(HID, _f32)
        for c in range(4):
            v[c * 128:(c + 1) * 128] = xs_t[:, li * 4 + c]
        xs.append(v)
    return xs


def total_exec_ns():
    return sum(v for k, v in _EXEC_NS if k == "kernel")


def exec_breakdown():
    return list(_EXEC_NS)


# revision 24
# speedup vs baseline: 3.7331x; 1.8911x over previous
"""ASAP-GNN classifier on trn2 via Bass/Tile.

Architecture (v3): ONE fused device program, single launch, no host
round-trips inside the network. Everything — 3x (GraphConv + ASAPool
attention + LEConv fitness), top-k node selection, kNN graph
construction, gather-index construction, and the per-layer global max —
runs on one NeuronCore inside one NEFF built around hardware For_i
loops (small program => fast build + AOT).

Key ideas vs v2 (which used 3 programs and 6 launches):
  * top-k selection is replaced by an on-device threshold bisection on
    the fitness logits z: ~44 For_i iterations of count(z > t) -> exact
    top-k mask (no compaction; unselected nodes are masked with -3e30
    and keep their slots, all phases stay at 157 row-blocks).
  * the kNN graph is computed on device as a masked dense distance GEMM
    over all 20096 slots (mask folded into a 5th GEMM row), followed by
    two rounds of max8/max_index -> 16 candidates per node.
  * dma_gather index tiles for the kNN layers are built ON DEVICE from
    the candidate table via tensor-engine transposes (fp32) + int16
    cast + 8-group replication.
  * per-layer global max is masked-max over all slots, reduced on
    device via transposes; host only runs the final 1x1536 MLP.

Host does: layer-0 slot-table construction from edge_index, one batched
upload (~11MB), one launch, one 6KB download, final MLP.
"""

import math
import threading
import time
import numpy as np

N0 = 20000
IN_CH = 64
HID = 512
OUT = 10
L = 3
RATIO = 0.5

_f32 = np.float32

# ---- geometry constants ----
NB = 157                    # row blocks
R = NB * 128                # 20096
ROWS0 = R + 128             # x0 rows (sentinel row = R, zeros)
ROWS = 20608                # h/a/xw rows (>= CCOL pad, gather-safe)
SENT = R                    # sentinel row id
D0C_DEFAULT = 17            # layer-0 max in-degree (rebuilt if differs)
K1, K2 = 6, 8               # kNN k for layers 1, 2
CH = 40                     # candidate chunks of 512
CCOL = CH * 512             # 20480 candidate columns (padded)
BIS = 44                    # bisection iterations
KTGT = [10000.0, 5000.0, 2500.0]


# ----------------------------------------------------------------------------
# bass plumbing
# ----------------------------------------------------------------------------

_BASS = {}


def _get_bass():
    if not _BASS:
        import concourse.bass as bass
        import concourse.bacc as bacc
        import concourse.mybir as mybir
        from concourse.tile import TileContext
        from concourse.masks import make_identity
        from concourse.bass import ds
        from concourse import bass2jax
        import jax
        import jax.numpy as jnp
        try:
            jax.config.update("jax_compilation_cache_dir",
                              "/tmp/jax_nc_cache")
            jax.config.update("jax_persistent_cache_min_entry_size_bytes", -1)
            jax.config.update("jax_persistent_cache_min_compile_time_secs", 0.1)
        except Exception:
            pass
        bass2jax.install_neuronx_cc_hook()
        _BASS.update(bass=bass, bacc=bacc, mybir=mybir, TileContext=TileContext,
                     make_identity=make_identity, ds=ds, bass2jax=bass2jax,
                     jax=jax, jnp=jnp)
    return _BASS


class _NcShim:
    """Stands in for a compiled Bacc during jax lowering: the lowering rule
    touches only to_json_bytes(), m.arch, has_collectives and
    target_bir_lowering."""

    target_bir_lowering = False

    def __init__(self, arch, has_collectives, json_bytes):
        import types
        self.has_collectives = has_collectives
        self._jb = json_bytes
        self.m = types.SimpleNamespace(arch=arch)

    def to_json_bytes(self):
        return self._jb


def _meta_from_nc(nc):
    B = _get_bass()
    mybir = B["mybir"]
    partition_name = (nc.partition_id_tensor.name
                      if nc.partition_id_tensor else None)
    in_names, in_specs, out_names, out_specs = [], [], [], []
    for alloc in nc.m.functions[0].allocations:
        if not isinstance(alloc, mybir.MemoryLocationSet):
            continue
        name = alloc.memorylocations[0].name
        if alloc.kind == "ExternalInput":
            if name != partition_name:
                in_names.append(name)
                in_specs.append((tuple(alloc.tensor_shape),
                                 np.dtype(mybir.dt.np(alloc.dtype)).str))
        elif alloc.kind == "ExternalOutput":
            out_names.append(name)
            out_specs.append((tuple(alloc.tensor_shape),
                              np.dtype(mybir.dt.np(alloc.dtype)).str))
    return {"in_names": in_names, "in_specs": in_specs,
            "out_names": out_names, "out_specs": out_specs,
            "partition_name": partition_name, "arch": nc.m.arch,
            "has_collectives": bool(nc.has_collectives),
            "json_bytes": nc.to_json_bytes(), "ncobj": nc}


class _Launcher:
    """Compiled 1-core bass program; inputs/outputs stay jax device arrays."""

    def __init__(self, meta):
        B = _get_bass()
        jax, jnp = B["jax"], B["jnp"]
        bass2jax = B["bass2jax"]
        ncobj = meta.get("ncobj")
        if ncobj is None:
            ncobj = _NcShim(meta["arch"], meta["has_collectives"],
                            meta["json_bytes"])
        partition_name = meta["partition_name"]
        self.in_names = list(meta["in_names"])
        self.in_avals = [jax.ShapeDtypeStruct(s, np.dtype(d))
                         for s, d in meta["in_specs"]]
        self.out_names = list(meta["out_names"])
        self.out_avals = [jax.core.ShapedArray(s, np.dtype(d))
                          for s, d in meta["out_specs"]]
        out_avals = self.out_avals
        out_names = self.out_names
        n_params = len(self.in_names)
        all_names = self.in_names + out_names + (
            [partition_name] if partition_name else [])
        donate = tuple(range(n_params, n_params + len(out_names)))

        def _body(*args):
            operands = list(args)
            if partition_name is not None:
                operands.append(bass2jax.partition_id_tensor())
            outs = bass2jax._bass_exec_p.bind(
                *operands, out_avals=tuple(out_avals),
                in_names=tuple(all_names), out_names=tuple(out_names),
                lowering_input_output_aliases=(),
                sim_require_finite=True, sim_require_nnan=True, nc=ncobj)
            return tuple(outs)

        self._jit = jax.jit(_body, donate_argnums=donate, keep_unused=True)
        self._compiled = None

    def warm(self):
        """AOT-compile the executable (no execution)."""
        B = _get_bass()
        jax = B["jax"]
        out_structs = [jax.ShapeDtypeStruct(av.shape, av.dtype)
                       for av in self.out_avals]
        self._compiled = self._jit.lower(*self.in_avals,
                                         *out_structs).compile()
        self._zeros = None
        try:
            jnp = B["jnp"]
            self._zeros = [jnp.zeros(av.shape, av.dtype).block_until_ready()
                           for av in self.out_avals]
        except Exception:
            pass
        return self

    def __call__(self, in_map):
        B = _get_bass()
        jnp = B["jnp"]
        args = [in_map[nm] for nm in self.in_names]
        zeros = getattr(self, "_zeros", None)
        if zeros is None:
            zeros = [jnp.zeros(av.shape, av.dtype) for av in self.out_avals]
        self._zeros = None    # donated; single-shot
        fn = self._compiled if self._compiled is not None else self._jit
        outs = fn(*args, *zeros)
        return dict(zip(self.out_names, outs))


# ----------------------------------------------------------------------------
# host helpers
# ----------------------------------------------------------------------------

def _idx_to_i16_tile(idx_list):
    """Compact dma_gather idx tile [16, S]: element m -> partition m%16,
    col m//16. Replicated across the 8 Q7 groups on device."""
    n = len(idx_list)
    S = (n + 15) // 16
    a = np.full((S, 16), -1, np.int16)
    a.reshape(-1)[:n] = idx_list.astype(np.int16)
    return np.ascontiguousarray(a.T)


def _slot_table(src, dst, nrows, D, sentinel):
    """[nrows, D] slot table: row i lists srcs of i's in-edges, sentinel pad."""
    deg = np.bincount(dst, minlength=nrows).astype(np.int64)
    order = np.argsort(dst, kind="stable")
    ss = src[order]
    dsrt = dst[order]
    starts = np.zeros(nrows + 1, np.int64)
    np.cumsum(deg, out=starts[1:])
    slot = np.arange(len(dsrt)) - starts[dsrt]
    tbl = np.full((nrows, D), sentinel, np.int64)
    tbl[dsrt, slot] = ss
    return tbl, deg


def _slotmajor_list(tbl):
    """[rows, D] -> block-slot-major gather list (per 128-block, slot-major)."""
    rows, D = tbl.shape
    nb = rows // 128
    return np.ascontiguousarray(
        tbl.reshape(nb, 128, D).transpose(0, 2, 1)).reshape(-1)


# ----------------------------------------------------------------------------
# blob layout (single batched upload)
# ----------------------------------------------------------------------------

def _layout(D0C):
    """name -> (word_offset, words, shape, kind). kind: f4 or i2 (bitcast)."""
    D0P = D0C + 1
    SP0 = NB * 8 * D0P
    specs = [
        ("x0", (ROWS0, IN_CH), "f4"),
        ("pidx0", (16, SP0), "i2"),
        ("invdeg", (R, 1), "f4"),
        ("cnt", (R, 1), "f4"),
        ("padmask", (128, NB), "f4"),
        ("selfidx", (16, NB * 8), "f4"),
        ("qc", (9, CCOL), "f4"),
        ("wxm", (128, HID), "f4"),
        ("wr1", (128, 4 * HID), "f4"),
        ("wl1", (128, 4 * HID), "f4"),
        ("wr2", (128, 4 * HID), "f4"),
        ("wl2", (128, 4 * HID), "f4"),
        ("vecs", (18, HID), "f4"),
        ("sc", (128, 16), "f4"),
    ]
    out = {}
    off = 0
    for name, shape, kind in specs:
        n = int(np.prod(shape))
        words = n // 2 if kind == "i2" else n
        assert words * (2 if kind == "i2" else 1) == n
        out[name] = (off, words, shape, kind)
        off += words
    return out, off


# ----------------------------------------------------------------------------
# program builder
# ----------------------------------------------------------------------------

def _tree_sum(nc, view, n):
    """In-place binary-tree reduce over slot axis; result lands in slot 0."""
    w = n
    while w > 1:
        h = w // 2
        nc.vector.tensor_add(view(0, h), view(0, h), view(h, h))
        if w % 2:
            nc.vector.tensor_add(view(0, 1), view(0, 1), view(w - 1, 1))
        w = h


def _tree_max(nc, gview, oview, n):
    """Max over n slots of g into out tile (slot tile of n//2 width)."""
    h = n // 2
    nc.vector.tensor_max(oview(0, h), gview(0, h), gview(h, h))
    if n % 2:
        nc.vector.tensor_max(oview(0, 1), oview(0, 1), gview(n - 1, 1))
    w = h
    while w > 1:
        h2 = w // 2
        nc.vector.tensor_max(oview(0, h2), oview(0, h2), oview(h2, h2))
        if w % 2:
            nc.vector.tensor_max(oview(0, 1), oview(0, 1), oview(w - 1, 1))
        w = h2


def _build_F(D0C):
    """The fully fused 3-layer program."""
    B = _get_bass()
    bacc, mybir, TileContext = B["bacc"], B["mybir"], B["TileContext"]
    ds, make_identity = B["ds"], B["make_identity"]
    dt = mybir.dt
    AX = mybir.AxisListType.X
    OP = mybir.AluOpType
    ACT = mybir.ActivationFunctionType
    D0P = D0C + 1
    SP0B = 8 * D0P
    SP0 = NB * SP0B

    lay, total_words = _layout(D0C)

    nc = bacc.Bacc("TRN2", target_bir_lowering=False)
    blob_d = nc.dram_tensor("blob", [1, total_words], dt.float32,
                            kind="ExternalInput")
    x0_d = nc.dram_tensor("x0", [ROWS0, IN_CH], dt.float32, kind="Internal")
    pidx0_d = nc.dram_tensor("pidx0", [16, SP0], dt.int16, kind="Internal")
    invdeg_d = nc.dram_tensor("invdeg", [R, 1], dt.float32, kind="Internal")
    cnt_d = nc.dram_tensor("cnt", [R, 1], dt.float32, kind="Internal")
    padmask_d = nc.dram_tensor("padmask", [128, NB], dt.float32,
                               kind="Internal")
    selfidx_d = nc.dram_tensor("selfidx", [16, NB * 8], dt.float32,
                               kind="Internal")
    qc_d = nc.dram_tensor("qc", [9, CCOL], dt.float32, kind="Internal")
    wxm_d = nc.dram_tensor("wxm", [128, HID], dt.float32, kind="Internal")
    wr1_d = nc.dram_tensor("wr1", [128, 4, HID], dt.float32, kind="Internal")
    wl1_d = nc.dram_tensor("wl1", [128, 4, HID], dt.float32, kind="Internal")
    wr2_d = nc.dram_tensor("wr2", [128, 4, HID], dt.float32, kind="Internal")
    wl2_d = nc.dram_tensor("wl2", [128, 4, HID], dt.float32, kind="Internal")
    vecs_d = nc.dram_tensor("vecs", [18, HID], dt.float32, kind="Internal")
    sc_d = nc.dram_tensor("sc", [128, 16], dt.float32, kind="Internal")
    _scatter_dst = {
        "x0": x0_d[:, :], "pidx0": pidx0_d[:, :], "invdeg": invdeg_d[:, :],
        "cnt": cnt_d[:, :], "padmask": padmask_d[:, :],
        "selfidx": selfidx_d[:, :], "qc": qc_d[:, :], "wxm": wxm_d[:, :],
        "wr1": wr1_d[:, :, :].rearrange("p a c -> p (a c)"),
        "wl1": wl1_d[:, :, :].rearrange("p a c -> p (a c)"),
        "wr2": wr2_d[:, :, :].rearrange("p a c -> p (a c)"),
        "wl2": wl2_d[:, :, :].rearrange("p a c -> p (a c)"),
        "vecs": vecs_d[:, :], "sc": sc_d[:, :],
    }

    xs_d = nc.dram_tensor("xs", [128, 12], dt.float32, kind="ExternalOutput")

    h_d = nc.dram_tensor("fh", [ROWS, 576], dt.float32, kind="Internal")
    a_d = nc.dram_tensor("fa", [ROWS, 64], dt.float32, kind="Internal")
    zb_d = nc.dram_tensor("zb", [R, 1], dt.float32, kind="Internal")
    z_d = nc.dram_tensor("z", [R, 1], dt.float32, kind="Internal")
    zm_d = nc.dram_tensor("zm", [1, R], dt.float32, kind="Internal")
    xn_d = nc.dram_tensor("xn", [R, HID], dt.float32, kind="Internal")
    xw_d = nc.dram_tensor("xw", [ROWS, HID], dt.float32, kind="Internal")
    knn_d = nc.dram_tensor("knn", [R, 16], dt.uint16, kind="Internal")
    msk_d = nc.dram_tensor("msk", [1, R], dt.float32, kind="Internal")
    pad_d = nc.dram_tensor("padc", [1, CCOL - R], dt.float32, kind="Internal")

    with TileContext(nc) as tc:
        with tc.tile_pool(name="const", bufs=1) as cpool:
            # scatter the blob to the per-tensor internal DRAM buffers
            for nm, (off, words, shape, kind) in lay.items():
                src = blob_d[0:1, off:off + words]
                if kind == "i2":
                    src = src.bitcast(dt.int16)
                src = src.rearrange("one (r c) -> r (c one)", r=shape[0])
                nc.sync.dma_start(_scatter_dst[nm], src)

            ident = cpool.tile([128, 128], dt.float32)
            make_identity(nc, ident[:])
            ones_r = cpool.tile([1, 128], dt.float32)
            nc.vector.memset(ones_r[:], 1.0)
            idbig = cpool.tile([128, 128], dt.float32)
            nc.vector.tensor_scalar_mul(idbig[:], ident[:], 1e30)
            padmask_sb = cpool.tile([128, NB], dt.float32)
            nc.sync.dma_start(padmask_sb[:], padmask_d[:, :])
            sc_sb = cpool.tile([128, 16], dt.float32)
            nc.sync.dma_start(sc_sb[:], sc_d[:, :])
            xs_sb = cpool.tile([128, 12], dt.float32)
            runmax = cpool.tile([128, HID], dt.float32)
            nc.vector.memset(runmax[:], -1e30)
            zm_sb = cpool.tile([128, NB], dt.float32)
            masks = [cpool.tile([128, NB], dt.float32, name="mask%d" % i)
                     for i in range(L)]
            tlos = [cpool.tile([1, 1], dt.float32, name="tlo%d" % i)
                    for i in range(L)]
            tlo128s = [cpool.tile([128, 1], dt.float32, name="tlo128%d" % i)
                       for i in range(L)]
            # bisection state
            lohi = cpool.tile([1, 2], dt.float32)
            tcur = cpool.tile([1, 1], dt.float32)
            cnts = cpool.tile([1, 1], dt.float32)
            isgt = cpool.tile([1, 1], dt.float32)
            d1 = cpool.tile([1, 1], dt.float32)
            cntp = cpool.tile([128, 1], dt.float32)
            mn = cpool.tile([128, 1], dt.float32)
            mx = cpool.tile([128, 1], dt.float32)
            t1r = cpool.tile([1, 128], dt.float32)
            ones_c = cpool.tile([128, 1], dt.float32)
            nc.vector.memset(ones_c[:], 1.0)
            # sentinel rows
            srow = cpool.tile([1, 576], dt.float32)
            nc.vector.memset(srow[:], 0.0)
            nc.vector.memset(srow[:, 512:513], -1e30)
            nc.sync.dma_start(h_d[SENT:SENT + 1, :], srow[:])
            nc.sync.dma_start(a_d[SENT:SENT + 1, :], srow[:, 0:64])

            def load_vecs(pool, li):
                t = {}
                for j, nm in enumerate(
                        ["br", "wq", "aw2", "l1w", "l2w", "l3w"]):
                    v = pool.tile([128, HID], dt.float32, tag="v_" + nm)
                    nc.sync.dma_start(
                        v[:], vecs_d[li * 6 + j:li * 6 + j + 1, :]
                        .to_broadcast([128, HID]))
                    t[nm] = v
                t["qb"] = sc_sb[:, li * 3 + 0:li * 3 + 1]
                t["l1b"] = sc_sb[:, li * 3 + 1:li * 3 + 2]
                t["l3b"] = sc_sb[:, li * 3 + 2:li * 3 + 3]
                return t

            def pool_fit_phases(wpool, pspool, idx_sb, idxoff, DP, V, li,
                                cnt_imm):
                """ASAPool + LEConv fitness over DP slots (incl self).

                idx_sb: [128, *] int16 gather tile; per-block window at
                idxoff(i) with 8*DP cols. cnt_imm: None -> load cnt_d.
                """
                SPB = 8 * DP

                def pool_body(i):
                    g = wpool.tile([128, DP, 576], dt.float32, tag="g")
                    nc.gpsimd.dma_gather(
                        out_ap=g[:], in_ap=h_d[:, :],
                        idxs_ap=idx_sb[:, idxoff(i)],
                        num_idxs=128 * DP, num_idxs_reg=128 * DP,
                        elem_size=576, single_packet=False)
                    xq = wpool.tile([128, DP // 2, HID], dt.float32, tag="xq")
                    _tree_max(nc,
                              lambda lo, c: g[:, lo:lo + c, 0:HID],
                              lambda lo, c: xq[:, lo:lo + c, :], DP)
                    tmp = wpool.tile([128, HID], dt.float32, tag="tmp")
                    nc.vector.tensor_mul(tmp[:], xq[:, 0, :], V["wq"][:])
                    qs = wpool.tile([128, 1], dt.float32, tag="qs")
                    nc.vector.tensor_reduce(qs[:], tmp[:], axis=AX, op=OP.add)
                    nc.vector.tensor_add(qs[:], qs[:], V["qb"])
                    sc = wpool.tile([128, DP], dt.float32, tag="sc")
                    jsv = g[:, :, 512:513].squeeze(2)
                    nc.vector.tensor_scalar_add(sc[:], jsv, qs[:])
                    sc2 = wpool.tile([128, DP], dt.float32, tag="sc2")
                    nc.vector.tensor_scalar_mul(sc2[:], sc[:], 0.2)
                    nc.vector.tensor_max(sc[:], sc[:], sc2[:])
                    m = wpool.tile([128, 1], dt.float32, tag="m")
                    nc.vector.tensor_reduce(m[:], sc[:], axis=AX, op=OP.max)
                    nc.vector.tensor_scalar(sc[:], sc[:], m[:], None,
                                            op0=OP.subtract)
                    nc.scalar.activation(sc[:], sc[:], ACT.Exp)
                    ssum = wpool.tile([128, 1], dt.float32, tag="ssum")
                    nc.vector.tensor_reduce(ssum[:], sc[:], axis=AX, op=OP.add)
                    rec = wpool.tile([128, 1], dt.float32, tag="rec")
                    nc.vector.reciprocal(rec[:], ssum[:])
                    nc.vector.tensor_scalar_mul(sc[:], sc[:], rec[:])
                    gh = g[:, :, 0:HID]
                    nc.vector.tensor_mul(
                        gh, gh, sc[:].unsqueeze(2).to_broadcast(
                            [128, DP, HID]))
                    _tree_sum(nc, lambda lo, c: g[:, lo:lo + c, 0:HID], DP)
                    xn = g[:, 0, 0:HID]
                    nc.sync.dma_start(xn_d[ds(i * 128, 128), :], xn)
                    nc.vector.tensor_mul(tmp[:], xn, V["l1w"][:])
                    av = wpool.tile([128, 1], dt.float32, tag="av")
                    nc.vector.tensor_reduce(av[:], tmp[:], axis=AX, op=OP.add)
                    nc.sync.dma_start(a_d[ds(i * 128, 128), 0:1], av[:])
                    nc.vector.tensor_mul(tmp[:], xn, V["l2w"][:])
                    bv = wpool.tile([128, 1], dt.float32, tag="bv")
                    nc.vector.tensor_reduce(bv[:], tmp[:], axis=AX, op=OP.add)
                    nc.vector.tensor_mul(tmp[:], xn, V["l3w"][:])
                    cv = wpool.tile([128, 1], dt.float32, tag="cv")
                    nc.vector.tensor_reduce(cv[:], tmp[:], axis=AX, op=OP.add)
                    zb = wpool.tile([128, 1], dt.float32, tag="zb")
                    lb1 = wpool.tile([128, 1], dt.float32, tag="lb1")
                    if cnt_imm is None:
                        ct = wpool.tile([128, 1], dt.float32, tag="ct")
                        nc.sync.dma_start(ct[:], cnt_d[ds(i * 128, 128), :])
                        nc.vector.tensor_mul(zb[:], ct[:], bv[:])
                        nc.vector.tensor_mul(lb1[:], ct[:], V["l1b"])
                    else:
                        nc.vector.tensor_scalar_mul(zb[:], bv[:], cnt_imm)
                        nc.vector.tensor_scalar_mul(lb1[:], V["l1b"], cnt_imm)
                    nc.vector.tensor_sub(zb[:], cv[:], zb[:])
                    nc.vector.tensor_add(zb[:], zb[:], V["l3b"])
                    nc.vector.tensor_add(zb[:], zb[:], lb1[:])
                    nc.sync.dma_start(zb_d[ds(i * 128, 128), :], zb[:])

                with tc.For_i(0, NB) as i:
                    pool_body(i)

                def fit_body(i):
                    ga = wpool.tile([128, DP, 64], dt.float32, tag="ga")
                    nc.gpsimd.dma_gather(
                        out_ap=ga[:], in_ap=a_d[:, :],
                        idxs_ap=idx_sb[:, idxoff(i)],
                        num_idxs=128 * DP, num_idxs_reg=128 * DP,
                        elem_size=64, single_packet=False)
                    zs = wpool.tile([128, 1], dt.float32, tag="zs")
                    nc.vector.tensor_reduce(zs[:], ga[:, :, 0:1].squeeze(2),
                                            axis=AX, op=OP.add)
                    zbl = wpool.tile([128, 1], dt.float32, tag="zbl")
                    nc.sync.dma_start(zbl[:], zb_d[ds(i * 128, 128), :])
                    nc.vector.tensor_add(zs[:], zs[:], zbl[:])
                    nc.sync.dma_start(z_d[ds(i * 128, 128), :], zs[:])

                with tc.For_i(0, NB) as i:
                    fit_body(i)

            def thresh_phase(li, prevmask):
                """zm = z + prevmask; bisect to top-KTGT threshold."""
                with tc.tile_pool(name="tps%d" % li, bufs=1,
                                  space="PSUM") as pst:
                    nc.sync.dma_start(
                        zm_sb[:],
                        z_d[:, :].rearrange("(b p) one -> p (b one)", p=128))
                    nc.vector.tensor_add(zm_sb[:], zm_sb[:], prevmask[:])
                    nc.sync.dma_start(
                        zm_d[0:1, :].rearrange("one (b p) -> p (b one)",
                                               p=128), zm_sb[:])
                    # active min/max for lo/hi
                    neg = cpool.tile([128, NB], dt.float32, tag="neg%d" % li)
                    nc.vector.tensor_scalar_mul(neg[:], prevmask[:], -1.0)
                    nc.vector.tensor_add(neg[:], neg[:], zm_sb[:])
                    nc.vector.tensor_reduce(mn[:], neg[:], axis=AX, op=OP.min)
                    nc.vector.tensor_reduce(mx[:], zm_sb[:], axis=AX,
                                            op=OP.max)
                    tpr = pst.tile([1, 128], dt.float32, tag="tpr")
                    nc.tensor.transpose(tpr[:], mn[:], ident[:])
                    nc.vector.tensor_copy(t1r[:], tpr[:])
                    nc.vector.tensor_reduce(lohi[:, 0:1], t1r[:], axis=AX,
                                            op=OP.min)
                    nc.tensor.transpose(tpr[:], mx[:], ident[:])
                    nc.vector.tensor_copy(t1r[:], tpr[:])
                    nc.vector.tensor_reduce(lohi[:, 1:2], t1r[:], axis=AX,
                                            op=OP.max)
                    ktgt = KTGT[li]
                    tbp = pst.tile([128, 1], dt.float32, tag="tbp")
                    cnt1 = pst.tile([1, 1], dt.float32, tag="cnt1")
                    with tc.For_i(0, BIS) as it:
                        nc.vector.tensor_add(tcur[:], lohi[:, 0:1],
                                             lohi[:, 1:2])
                        nc.vector.tensor_scalar_mul(tcur[:], tcur[:], 0.5)
                        nc.tensor.matmul(tbp[:], ones_r[:], tcur[:],
                                         start=True, stop=True)
                        tbs = cpool.tile([128, 1], dt.float32, tag="tbs")
                        nc.vector.tensor_copy(tbs[:], tbp[:])
                        cmp = cpool.tile([128, NB], dt.float32, tag="cmp")
                        nc.vector.tensor_scalar(cmp[:], zm_sb[:], tbs[:],
                                                None, op0=OP.is_gt)
                        nc.vector.tensor_reduce(cntp[:], cmp[:], axis=AX,
                                                op=OP.add)
                        nc.tensor.matmul(cnt1[:], cntp[:], ones_c[:],
                                         start=True, stop=True)
                        nc.vector.tensor_copy(cnts[:], cnt1[:])
                        nc.vector.tensor_scalar(isgt[:], cnts[:], ktgt - 0.5,
                                                None, op0=OP.is_gt)
                        nc.vector.tensor_sub(d1[:], tcur[:], lohi[:, 0:1])
                        nc.vector.tensor_mul(d1[:], d1[:], isgt[:])
                        nc.vector.tensor_add(lohi[:, 0:1], lohi[:, 0:1],
                                             d1[:])
                        nc.vector.tensor_sub(d1[:], lohi[:, 1:2], tcur[:])
                        nc.vector.tensor_mul(d1[:], d1[:], isgt[:])
                        nc.vector.tensor_add(lohi[:, 1:2], tcur[:], d1[:])
                    nc.vector.tensor_copy(tlos[li][:], lohi[:, 0:1])
                    nc.tensor.matmul(tbp[:], ones_r[:], tlos[li][:],
                                     start=True, stop=True)
                    nc.vector.tensor_copy(tlo128s[li][:], tbp[:])
                    # maskadd = (zm > tlo ? 0 : -3e30)
                    nc.vector.tensor_scalar(masks[li][:], zm_sb[:],
                                            tlo128s[li][:], None,
                                            op0=OP.is_gt)
                    nc.vector.tensor_scalar_add(masks[li][:], masks[li][:],
                                                -1.0)
                    nc.vector.tensor_scalar_mul(masks[li][:], masks[li][:],
                                                3e30)

            def xw_xs_phase(li, write_xw):
                """xw = xn*sigmoid(zm) (-> xw_d), masked running max -> xs."""
                with (
                    tc.tile_pool(name="wp_w%d" % li, bufs=2) as wp,
                    tc.tile_pool(name="psw%d" % li, bufs=1,
                                 space="PSUM") as psw,
                ):
                    def w_body(i):
                        fv = wp.tile([128, 1], dt.float32, tag="fv")
                        nc.scalar.activation(fv[:], zm_sb[:, ds(i, 1)],
                                             ACT.Sigmoid)
                        xnb = wp.tile([128, HID], dt.float32, tag="xnb")
                        nc.sync.dma_start(xnb[:], xn_d[ds(i * 128, 128), :])
                        xw = wp.tile([128, HID], dt.float32, tag="xw")
                        nc.vector.tensor_scalar_mul(xw[:], xnb[:], fv[:])
                        if write_xw:
                            nc.sync.dma_start(xw_d[ds(i * 128, 128), :],
                                              xw[:])
                        nc.vector.tensor_scalar_add(
                            xw[:], xw[:], masks[li][:, ds(i, 1)])
                        nc.vector.tensor_max(runmax[:], runmax[:], xw[:])

                    with tc.For_i(0, NB) as i:
                        w_body(i)
                    # reduce runmax across partitions into xs_sb cols
                    for c in range(4):
                        tp = psw.tile([128, 128], dt.float32,
                                      tag="tp%d" % (c % 2))
                        nc.tensor.transpose(tp[:],
                                            runmax[:, c * 128:(c + 1) * 128],
                                            ident[:])
                        nc.vector.tensor_reduce(
                            xs_sb[:, li * 4 + c:li * 4 + c + 1], tp[:],
                            axis=AX, op=OP.max)
                    nc.vector.memset(runmax[:], -1e30)

            def knn_phase(li):
                """Masked dense kNN sweep -> knn_d (16 candidates/node)."""
                with (
                    tc.tile_pool(name="wp_k%d" % li, bufs=2) as wp,
                    tc.tile_pool(name="psk%d" % li, bufs=2,
                                 space="PSUM") as psk,
                ):
                    cand = wp.tile([5, CCOL], dt.float32, tag="cand", bufs=1)
                    nc.sync.dma_start(cand[0:4, :], qc_d[5:9, :])
                    row = wp.tile([128, CCOL], dt.float32, tag="row", bufs=1)
                    # mask row: reuse row[0:1] as scratch, then bounce via
                    # DRAM for the partition shift 0 -> 4
                    zmr = row[0:1, 0:R]
                    nc.sync.dma_start(zmr, zm_d[:, :])
                    nc.vector.tensor_scalar(zmr, zmr,
                                            tlos[li][:], None, op0=OP.is_gt)
                    nc.vector.tensor_scalar_add(zmr, zmr, -1.0)
                    nc.vector.tensor_scalar_mul(zmr, zmr, 3e30)
                    nc.sync.dma_start(msk_d[:, :], zmr)
                    nc.sync.dma_start(cand[4:5, 0:R], msk_d[:, :])
                    if CCOL > R:
                        padc = wp.tile([1, CCOL - R], dt.float32, tag="padc")
                        nc.vector.memset(padc[:], -3e30)
                        nc.sync.dma_start(pad_d[:, :], padc[:])
                        nc.sync.dma_start(cand[4:5, R:CCOL], pad_d[:, :])

                    HCOL = CCOL // 2

                    def k_body(i):
                        qsb = wp.tile([5, 128], dt.float32, tag="qsb")
                        nc.sync.dma_start(qsb[:], qc_d[0:5, ds(i * 128, 128)])
                        for ch in range(CH):
                            dps = psk.tile([128, 512], dt.float32,
                                           tag="d%d" % (ch % 2))
                            nc.tensor.matmul(dps[:], qsb[:],
                                             cand[:, ch * 512:(ch + 1) * 512],
                                             start=True, stop=True)
                            nc.scalar.activation(
                                row[:, ch * 512:(ch + 1) * 512], dps[:],
                                ACT.Copy)
                        # self-exclusion on the diagonal block
                        nc.vector.tensor_sub(row[:, ds(i * 128, 128)],
                                             row[:, ds(i * 128, 128)],
                                             idbig[:])
                        # per-half top-8 (max8 input cap is 16384)
                        vab = wp.tile([128, 16], dt.float32, tag="vab")
                        iab = wp.tile([128, 16], dt.float32, tag="iab")
                        vA = wp.tile([128, 8], dt.float32, tag="vA")
                        iA = wp.tile([128, 8], dt.uint32, tag="iA")
                        nc.vector.max(out=vA[:], in_=row[:, 0:HCOL])
                        nc.vector.max_index(iA[:], vA[:], row[:, 0:HCOL])
                        nc.vector.tensor_copy(vab[:, 0:8], vA[:])
                        nc.vector.tensor_copy(iab[:, 0:8], iA[:])
                        vB = wp.tile([128, 8], dt.float32, tag="vB")
                        iB = wp.tile([128, 8], dt.uint32, tag="iB")
                        nc.vector.max(out=vB[:], in_=row[:, HCOL:CCOL])
                        nc.vector.max_index(iB[:], vB[:], row[:, HCOL:CCOL])
                        nc.vector.tensor_copy(vab[:, 8:16], vB[:])
                        nc.vector.tensor_copy(iab[:, 8:16], iB[:])
                        nc.vector.tensor_scalar_add(iab[:, 8:16],
                                                    iab[:, 8:16],
                                                    float(HCOL))
                        # iab1 = idx + 1 (0 must not survive the eq-mask max)
                        nc.vector.tensor_scalar_add(iab[:], iab[:], 1.0)
                        # merge the two sorted top-8 lists -> top-10 distinct
                        outf = wp.tile([128, 16], dt.float32, tag="outf")
                        mm = wp.tile([128, 1], dt.float32, tag="mm")
                        eq = wp.tile([128, 16], dt.float32, tag="eq")
                        tmq = wp.tile([128, 16], dt.float32, tag="tmq")
                        for s in range(10):
                            nc.vector.tensor_reduce(mm[:], vab[:], axis=AX,
                                                    op=OP.max)
                            nc.vector.tensor_scalar(eq[:], vab[:], mm[:],
                                                    None, op0=OP.is_equal)
                            nc.vector.tensor_mul(tmq[:], eq[:], iab[:])
                            nc.vector.tensor_reduce(outf[:, s:s + 1], tmq[:],
                                                    axis=AX, op=OP.max)
                            nc.vector.tensor_scalar_mul(tmq[:], eq[:], -1e31)
                            nc.vector.tensor_add(vab[:], vab[:], tmq[:])
                        nc.vector.tensor_scalar_add(outf[:, 0:10],
                                                    outf[:, 0:10], -1.0)
                        i16 = wp.tile([128, 16], dt.uint16, tag="i16")
                        nc.vector.tensor_copy(i16[:, 0:10], outf[:, 0:10])
                        nc.sync.dma_start(knn_d[ds(i * 128, 128), 0:10],
                                          i16[:, 0:10])

                    with tc.For_i(0, NB) as i:
                        k_body(i)

            def idx_build_phase(pool, li, D):
                """Build [128, NB*(D+1)*8] int16 gather tile from knn_d."""
                W = (D + 1) * 8
                idxg = pool.tile([128, NB * W], dt.int16, tag="idxg")
                with (
                    tc.tile_pool(name="ib%d" % li, bufs=2) as ib,
                    tc.tile_pool(name="psib%d" % li, bufs=2,
                                 space="PSUM") as psib,
                ):
                    stage = ib.tile([16, NB, D + 1, 8], dt.float32,
                                    tag="stage", bufs=1)
                    nc.sync.dma_start(
                        stage[:, :, D, :],
                        selfidx_d[:, :].rearrange("p (b c) -> p b c", b=NB))

                    def ib_body(i):
                        kb = ib.tile([128, 16], dt.uint16, tag="kb")
                        nc.sync.dma_start(kb[:], knn_d[ds(i * 128, 128), :])
                        kf = ib.tile([128, 16], dt.float32, tag="kf")
                        nc.vector.tensor_copy(kf[:], kb[:])
                        t1ps = psib.tile([16, 128], dt.float32, tag="t1ps")
                        nc.tensor.transpose(t1ps[:], kf[:], ident[:])
                        t1t = ib.tile([16, 128], dt.float32, tag="t1t")
                        nc.vector.tensor_copy(t1t[:], t1ps[:])
                        for c8 in range(8):
                            t2ps = psib.tile([16, 16], dt.float32,
                                             tag="t2_%d" % (c8 % 2))
                            nc.tensor.transpose(t2ps[:],
                                                t1t[:, c8 * 16:(c8 + 1) * 16],
                                                ident[0:16, 0:16])
                            nc.vector.tensor_copy(
                                stage[:, ds(i, 1), 0:D, c8].squeeze(1),
                                t2ps[:, 0:D])

                    with tc.For_i(0, NB) as i:
                        ib_body(i)
                    idx16 = ib.tile([16, NB * W], dt.int16, tag="idx16",
                                    bufs=1)
                    nc.vector.tensor_copy(
                        idx16[:],
                        stage[:].rearrange("p a b c -> p (a b c)"))
                    for g in range(8):
                        nc.sync.dma_start(idxg[g * 16:(g + 1) * 16, :],
                                          idx16[:])
                return idxg

            def conv_knn_phase(wpool, pspool, idxg, D, V, wr_t, wl_t):
                """GraphConv over the kNN graph (D neighbors + self slot)."""
                DP = D + 1
                WB = DP * 8

                def c_body(i):
                    g = wpool.tile([128, DP, HID], dt.float32, tag="cg")
                    nc.gpsimd.dma_gather(
                        out_ap=g[:], in_ap=xw_d[:, :],
                        idxs_ap=idxg[:, ds(i * WB, WB)],
                        num_idxs=128 * DP, num_idxs_reg=128 * DP,
                        elem_size=HID, single_packet=False)
                    _tree_sum(nc, lambda lo, c: g[:, lo:lo + c, :], D)
                    mean = wpool.tile([128, HID], dt.float32, tag="mean")
                    nc.vector.tensor_scalar_mul(mean[:], g[:, 0, :], 1.0 / D)
                    hps = pspool.tile([128, HID], dt.float32, tag="hps")
                    xt = wpool.tile([128, 4, 128], dt.float32, tag="xt")
                    mt = wpool.tile([128, 4, 128], dt.float32, tag="mt")
                    for c in range(4):
                        tp = pspool.tile([128, 128], dt.float32,
                                         tag="tp%d" % (c % 2))
                        nc.tensor.transpose(tp[:],
                                            g[:, D, c * 128:(c + 1) * 128],
                                            ident[:])
                        nc.vector.tensor_copy(xt[:, c, :], tp[:])
                        tp2 = pspool.tile([128, 128], dt.float32,
                                          tag="tq%d" % (c % 2))
                        nc.tensor.transpose(tp2[:],
                                            mean[:, c * 128:(c + 1) * 128],
                                            ident[:])
                        nc.vector.tensor_copy(mt[:, c, :], tp2[:])
                    for c in range(4):
                        nc.tensor.matmul(hps[:], xt[:, c, :], wl_t[:, c, :],
                                         start=(c == 0), stop=False)
                    for c in range(4):
                        nc.tensor.matmul(hps[:], mt[:, c, :], wr_t[:, c, :],
                                         start=False, stop=(c == 3))
                    hsb = wpool.tile([128, 576], dt.float32, tag="hsb")
                    nc.vector.tensor_add(hsb[:, 0:HID], hps[:], V["br"][:])
                    nc.vector.tensor_scalar_max(hsb[:, 0:HID], hsb[:, 0:HID],
                                                0.0)
                    tmp = wpool.tile([128, HID], dt.float32, tag="ctmp")
                    nc.vector.tensor_mul(tmp[:], hsb[:, 0:HID], V["aw2"][:])
                    nc.vector.tensor_reduce(hsb[:, 512:513], tmp[:], axis=AX,
                                            op=OP.add)
                    nc.sync.dma_start(h_d[ds(i * 128, 128), 0:513],
                                      hsb[:, 0:513])

                with tc.For_i(0, NB) as i:
                    c_body(i)

            # ================= layer 0 =================
            with tc.tile_pool(name="seg0", bufs=1) as seg0:
                pidx0_sb = seg0.tile([128, SP0], dt.int16, tag="pidx0")
                for g in range(8):
                    nc.sync.dma_start(pidx0_sb[g * 16:(g + 1) * 16, :],
                                      pidx0_d[:, :])
                V0 = load_vecs(seg0, 0)
                wxm_sb = seg0.tile([128, HID], dt.float32, tag="wxm")
                nc.sync.dma_start(wxm_sb[:], wxm_d[:, :])

                with (
                    tc.tile_pool(name="l0c", bufs=2) as wp0,
                    tc.tile_pool(name="ps0", bufs=2, space="PSUM") as ps0,
                ):
                    def conv0_body(i):
                        g = wp0.tile([128, D0P, IN_CH], dt.float32, tag="g0")
                        nc.gpsimd.dma_gather(
                            out_ap=g[:], in_ap=x0_d[:, :],
                            idxs_ap=pidx0_sb[:, ds(i * SP0B, SP0B)],
                            num_idxs=128 * D0P, num_idxs_reg=128 * D0P,
                            elem_size=IN_CH, single_packet=False)
                        _tree_sum(nc, lambda lo, c: g[:, lo:lo + c, :], D0P)
                        xm = wp0.tile([128, 128], dt.float32, tag="xm")
                        nc.sync.dma_start(xm[:, 0:IN_CH],
                                          x0_d[ds(i * 128, 128), :])
                        # neighbors-only sum = sum(all slots) - self
                        nc.vector.tensor_sub(g[:, 0, :], g[:, 0, :],
                                             xm[:, 0:IN_CH])
                        iv = wp0.tile([128, 1], dt.float32, tag="iv")
                        nc.sync.dma_start(iv[:],
                                          invdeg_d[ds(i * 128, 128), :])
                        nc.vector.tensor_scalar_mul(xm[:, IN_CH:2 * IN_CH],
                                                    g[:, 0, :], iv[:])
                        tp = ps0.tile([128, 128], dt.float32, tag="tp0")
                        nc.tensor.transpose(tp[:], xm[:], ident[:])
                        lhsT = wp0.tile([128, 128], dt.float32, tag="lhsT")
                        nc.vector.tensor_copy(lhsT[:], tp[:])
                        hps = ps0.tile([128, HID], dt.float32, tag="hps0")
                        nc.tensor.matmul(hps[:], lhsT[:], wxm_sb[:],
                                         start=True, stop=True)
                        hsb = wp0.tile([128, 576], dt.float32, tag="hsb0")
                        nc.vector.tensor_add(hsb[:, 0:HID], hps[:],
                                             V0["br"][:])
                        nc.vector.tensor_scalar_max(hsb[:, 0:HID],
                                                    hsb[:, 0:HID], 0.0)
                        tmp = wp0.tile([128, HID], dt.float32, tag="tmp0")
                        nc.vector.tensor_mul(tmp[:], hsb[:, 0:HID],
                                             V0["aw2"][:])
                        nc.vector.tensor_reduce(hsb[:, 512:513], tmp[:],
                                                axis=AX, op=OP.add)
                        nc.sync.dma_start(h_d[ds(i * 128, 128), 0:513],
                                          hsb[:, 0:513])

                    with tc.For_i(0, NB) as i:
                        conv0_body(i)
                with (
                    tc.tile_pool(name="l0p", bufs=2) as wp0p,
                    tc.tile_pool(name="ps0p", bufs=2, space="PSUM") as ps0p,
                ):
                    pool_fit_phases(wp0p, ps0p, pidx0_sb,
                                    lambda i: ds(i * SP0B, SP0B), D0P, V0, 0,
                                    None)
            thresh_phase(0, padmask_sb)
            xw_xs_phase(0, True)
            knn_phase(0)

            # ================= layer 1 =================
            with tc.tile_pool(name="seg1", bufs=1) as seg1:
                idxg1 = idx_build_phase(seg1, 1, K1)
                with (
                    tc.tile_pool(name="l1", bufs=2) as wp1,
                    tc.tile_pool(name="ps1", bufs=1, space="PSUM") as ps1,
                ):
                    V1 = load_vecs(wp1, 1)
                    wr1_sb = wp1.tile([128, 4, HID], dt.float32, tag="wr",
                                      bufs=1)
                    nc.sync.dma_start(wr1_sb[:], wr1_d[:, :, :])
                    wl1_sb = wp1.tile([128, 4, HID], dt.float32, tag="wl",
                                      bufs=1)
                    nc.sync.dma_start(wl1_sb[:], wl1_d[:, :, :])
                    conv_knn_phase(wp1, ps1, idxg1, K1, V1, wr1_sb, wl1_sb)
                    pool_fit_phases(wp1, ps1, idxg1,
                                    lambda i: ds(i * (K1 + 1) * 8,
                                                 (K1 + 1) * 8),
                                    K1 + 1, V1, 1, float(K1 + 1))
            thresh_phase(1, masks[0])
            xw_xs_phase(1, True)
            knn_phase(1)

            # ================= layer 2 =================
            with tc.tile_pool(name="seg2", bufs=1) as seg2:
                idxg2 = idx_build_phase(seg2, 2, K2)
                with (
                    tc.tile_pool(name="l2", bufs=2) as wp2,
                    tc.tile_pool(name="ps2", bufs=1, space="PSUM") as ps2,
                ):
                    V2 = load_vecs(wp2, 2)
                    wr2_sb = wp2.tile([128, 4, HID], dt.float32, tag="wr",
                                      bufs=1)
                    nc.sync.dma_start(wr2_sb[:], wr2_d[:, :, :])
                    wl2_sb = wp2.tile([128, 4, HID], dt.float32, tag="wl",
                                      bufs=1)
                    nc.sync.dma_start(wl2_sb[:], wl2_d[:, :, :])
                    conv_knn_phase(wp2, ps2, idxg2, K2, V2, wr2_sb, wl2_sb)
                    pool_fit_phases(wp2, ps2, idxg2,
                                    lambda i: ds(i * (K2 + 1) * 8,
                                                 (K2 + 1) * 8),
                                    K2 + 1, V2, 2, float(K2 + 1))
            thresh_phase(2, masks[1])
            xw_xs_phase(2, False)

            nc.sync.dma_start(xs_d[:, :], xs_sb[:])
    nc.compile()
    return nc


# ----------------------------------------------------------------------------
# build/compile management (import-time warm-up)
# ----------------------------------------------------------------------------

_RUNNERS = {}
_BUILD_LOCK = threading.Lock()
_BUILD_THREADS = []
_READY = {"F_%d" % D0C_DEFAULT: threading.Event()}
_CACHE_DIR = "/tmp/asap_gnn_v3_cache"


def _cache_path(D0C):
    import hashlib
    import inspect
    try:
        srcs = inspect.getsource(_build_F)
    except Exception:
        srcs = "nosrc"
    key = "%s|%d|%d|%d|%d|%d|%d|%s" % (
        srcs, D0C, NB, CCOL, BIS, K1, K2, KTGT)
    h = hashlib.sha1(key.encode()).hexdigest()[:16]
    return "%s/F_%d_%s.pkl" % (_CACHE_DIR, D0C, h)


def _load_meta(D0C):
    import pickle
    try:
        with open(_cache_path(D0C), "rb") as f:
            return pickle.load(f)
    except Exception:
        return None


def _save_meta(D0C, meta):
    import os
    import pickle
    try:
        os.makedirs(_CACHE_DIR, exist_ok=True)
        d = {k: v for k, v in meta.items() if k != "ncobj"}
        tmp = _cache_path(D0C) + ".tmp.%d" % os.getpid()
        with open(tmp, "wb") as f:
            pickle.dump(d, f)
        os.replace(tmp, _cache_path(D0C))
    except Exception:
        pass


def _make_launcher(D0C):
    meta = _load_meta(D0C)
    if meta is None:
        meta = _meta_from_nc(_build_F(D0C))
        _save_meta(D0C, meta)
    return _Launcher(meta).warm()


def _get_runner(name, D0C):
    with _BUILD_LOCK:
        if name in _RUNNERS:
            return _RUNNERS[name]
    r = _make_launcher(D0C)
    with _BUILD_LOCK:
        _RUNNERS.setdefault(name, r)
    return _RUNNERS[name]


_WARM_NS = []


def _warm():
    t00 = time.perf_counter()

    def _mark(lbl):
        _WARM_NS.append((lbl, int((time.perf_counter() - t00) * 1e9)))

    try:
        jini = threading.Thread(target=lambda: _get_bass()["jax"].devices())
        jini.start()
        name = "F_%d" % D0C_DEFAULT
        l = _make_launcher(D0C_DEFAULT)
        _mark("w_launcher")
        with _BUILD_LOCK:
            _RUNNERS.setdefault(name, l)
        jini.join()
        _mark("w_jax")
        try:
            B = _get_bass()
            jax = B["jax"]
            dev = jax.devices()[0]
            jax.device_put(np.zeros((1, 16), np.float32),
                           dev).block_until_ready()
        except Exception:
            pass
        _mark("w_tinyput")
    except Exception:  # pragma: no cover - fallback path handles
        import traceback
        traceback.print_exc()
    finally:
        for ev in _READY.values():
            ev.set()


_BUILD_THREADS.append(threading.Thread(target=_warm, daemon=True))
_BUILD_THREADS[-1].start()


# ----------------------------------------------------------------------------
# numpy fallback (used only if the device path fails)
# ----------------------------------------------------------------------------

def _np_reference(x, pos, src, dst, W):
    f = _f32
    n = N0
    xs = []
    for i in range(L):
        wr, br, wl = W["wr"][i], W["br"][i], W["wl"][i]
        agg = np.zeros((n, x.shape[1]), f)
        np.add.at(agg, dst, x[src])
        deg = np.bincount(dst, minlength=n).astype(f)
        mean = agg / np.maximum(deg, 1)[:, None]
        h = np.maximum(mean @ wr + br + x @ wl, 0).astype(f)
        sl = np.arange(n)
        s_ = np.concatenate([src, sl])
        d_ = np.concatenate([dst, sl])
        xj = h[s_]
        xq = np.full((n, HID), -np.inf, f)
        np.maximum.at(xq, d_, xj)
        xq = (xq @ W["lw"][i] + W["lb"][i]).astype(f)
        aw, ab = W["aw"][i], W["ab"][i]
        score = (xq[d_] @ aw[:HID] + xj @ aw[HID:] + ab).astype(f)
        score = np.where(score > 0, score, f(0.2) * score).astype(f)
        smax = np.full(n, -np.inf, f)
        np.maximum.at(smax, d_, score)
        ex = np.exp(score - smax[d_])
        ssum = np.zeros(n, f)
        np.add.at(ssum, d_, ex)
        att = (ex / ssum[d_]).astype(f)
        xn = np.zeros((n, HID), f)
        np.add.at(xn, d_, xj * att[:, None])
        a = xn @ W["l1w"][i] + W["l1b"][i]
        b = xn @ W["l2w"][i]
        agg2 = np.zeros(n, f)
        np.add.at(agg2, d_, (a[s_] - b[d_]).astype(f))
        z = (agg2 + xn @ W["l3w"][i] + W["l3b"][i]).astype(f)
        k_keep = int(math.ceil(RATIO * n))
        fit64 = 1.0 / (1.0 + np.exp(-z.astype(np.float64)))
        perm = np.argpartition(-fit64, k_keep - 1)[:k_keep]
        fv = fit64[perm].astype(f)
        x = (xn[perm] * fv[:, None]).astype(f)
        xs.append(x.max(0))
        pos = pos[perm]
        n = k_keep
        if i < L - 1:
            k = 6 + 2 * i
            sq = np.sum(pos * pos, -1)
            dist = sq[:, None] + sq[None, :] - 2 * (pos @ pos.T)
            np.fill_diagonal(dist, np.inf)
            idx = np.argpartition(dist, k, 1)[:, :k]
            srt = np.take_along_axis(dist, idx, 1).argsort(1, kind="stable")
            idx = np.take_along_axis(idx, srt, 1)
            dst = np.repeat(np.arange(n), k)
            src = idx.reshape(-1)
    return xs


# ----------------------------------------------------------------------------
# main kernel
# ----------------------------------------------------------------------------

_EXEC_NS = []


def kernel(x, pos, edge_index, conv0_wr, conv0_br, conv0_wl, conv_wr, conv_br,
           conv_wl, pool_lin_w, pool_lin_b, pool_att_w, pool_att_b, le1_w,
           le1_b, le2_w, le3_w, le3_b, lin1_w, lin1_b, lin2_w, lin2_b):
    t_start = time.perf_counter()
    _EXEC_NS.clear()
    x = np.asarray(x, _f32)
    pos = np.asarray(pos, _f32)
    ei = np.asarray(edge_index).astype(np.int64)

    W = {
        "wr": [np.asarray(conv0_wr, _f32)] + [np.asarray(conv_wr[i], _f32)
                                              for i in range(L - 1)],
        "br": [np.asarray(conv0_br, _f32)] + [np.asarray(conv_br[i], _f32)
                                              for i in range(L - 1)],
        "wl": [np.asarray(conv0_wl, _f32)] + [np.asarray(conv_wl[i], _f32)
                                              for i in range(L - 1)],
        "lw": [np.asarray(pool_lin_w[i], _f32) for i in range(L)],
        "lb": [np.asarray(pool_lin_b[i], _f32) for i in range(L)],
        "aw": [np.asarray(pool_att_w[i], _f32) for i in range(L)],
        "ab": [float(pool_att_b[i]) for i in range(L)],
        "l1w": [np.asarray(le1_w[i], _f32) for i in range(L)],
        "l1b": [float(le1_b[i]) for i in range(L)],
        "l2w": [np.asarray(le2_w[i], _f32) for i in range(L)],
        "l3w": [np.asarray(le3_w[i], _f32) for i in range(L)],
        "l3b": [float(le3_b[i]) for i in range(L)],
    }
    try:
        xs = _device_forward(x, pos, ei, W)
    except Exception:
        import traceback
        traceback.print_exc()
        print("kernel: device path failed; numpy fallback")
        xs = _np_reference(x, pos, ei[0], ei[1], W)

    hcat = np.concatenate(xs)[None, :].astype(_f32)
    h1 = np.maximum(hcat @ np.asarray(lin1_w, _f32) +
                    np.asarray(lin1_b, _f32), 0)
    out = (h1 @ np.asarray(lin2_w, _f32) + np.asarray(lin2_b, _f32))
    dt_ns = int((time.perf_counter() - t_start) * 1e9)
    _EXEC_NS.append(("kernel", dt_ns))
    return out.astype(_f32)


def _device_forward(x, pos, ei, W):
    _T0 = time.perf_counter()
    src, dst = ei[0], ei[1]

    # ---------------- host prep (pure numpy, overlaps warm) ----------------
    deg0 = np.bincount(dst, minlength=R).astype(np.int64)
    D0C = max(int(deg0.max()), 1)
    name = "F_%d" % D0C

    x0 = np.zeros((ROWS0, IN_CH), _f32)
    x0[:N0] = x
    tblC, _ = _slot_table(src, dst, R, D0C, SENT)
    tblP = np.concatenate(
        [np.arange(R, dtype=np.int64)[:, None], tblC], 1)
    tblP[N0:, 0] = SENT
    pidx0 = _idx_to_i16_tile(_slotmajor_list(tblP))
    invdeg0 = (1.0 / np.maximum(deg0, 1.0)).astype(_f32)[:, None]
    cnt0 = (deg0 + 1).astype(_f32)[:, None]
    padmask = np.zeros((128, NB), _f32)
    for j in range(N0, R):
        padmask[j % 128, j // 128] = -3e30
    selfidx = np.zeros((16, NB * 8), _f32)
    ar = np.arange(R).reshape(NB, 8, 16)           # [b, c8, p16]
    selfidx[:, :] = ar.transpose(2, 0, 1).reshape(16, NB * 8)
    sq = np.sum(pos * pos, 1, dtype=_f32)
    qc = np.zeros((9, CCOL), _f32)
    qc[0, :N0] = 2.0 * pos[:, 0]
    qc[1, :N0] = 2.0 * pos[:, 1]
    qc[2, :N0] = -1.0
    qc[3, :N0] = -sq
    qc[4, :N0] = 1.0
    qc[5, :N0] = pos[:, 0]
    qc[6, :N0] = pos[:, 1]
    qc[7, :N0] = sq
    qc[7, N0:] = 1e30
    qc[8, :] = 1.0
    wxm = np.zeros((128, HID), _f32)
    wxm[0:IN_CH] = W["wl"][0]
    wxm[IN_CH:2 * IN_CH] = W["wr"][0]
    vecs = np.zeros((18, HID), _f32)
    sc = np.zeros((128, 16), _f32)
    for li in range(L):
        lw, lb = W["lw"][li], W["lb"][li]
        aw, ab = W["aw"][li], W["ab"][li]
        wq = (lw @ aw[:HID]).astype(_f32)
        qb = float(lb @ aw[:HID] + ab)
        vecs[li * 6 + 0] = W["br"][li] if li == 0 else W["br"][li]
        vecs[li * 6 + 1] = wq
        vecs[li * 6 + 2] = aw[HID:]
        vecs[li * 6 + 3] = W["l1w"][li]
        vecs[li * 6 + 4] = W["l2w"][li]
        vecs[li * 6 + 5] = W["l3w"][li]
        sc[:, li * 3 + 0] = qb
        sc[:, li * 3 + 1] = W["l1b"][li]
        sc[:, li * 3 + 2] = W["l3b"][li]
    wpack = {}
    for li in (1, 2):
        wpack["wr%d" % li] = np.ascontiguousarray(
            W["wr"][li].reshape(4, 128, HID).transpose(1, 0, 2))
        wpack["wl%d" % li] = np.ascontiguousarray(
            W["wl"][li].reshape(4, 128, HID).transpose(1, 0, 2))

    host = {"x0": x0, "pidx0": pidx0, "invdeg": invdeg0, "cnt": cnt0,
            "padmask": padmask, "selfidx": selfidx, "qc": qc, "wxm": wxm,
            "wr1": wpack["wr1"], "wl1": wpack["wl1"], "wr2": wpack["wr2"],
            "wl2": wpack["wl2"], "vecs": vecs, "sc": sc}
    lay, total_words = _layout(D0C)
    blob = np.empty((1, total_words), _f32)
    for nm, (off, words, shape, kind) in lay.items():
        a = host[nm]
        if kind == "i2":
            blob[0, off:off + words] = np.ascontiguousarray(
                a).reshape(-1).view(_f32)
        else:
            blob[0, off:off + words] = a.reshape(-1)
    _EXEC_NS.append(("prep", int((time.perf_counter() - _T0) * 1e9)))

    # ---------------- wait for warm first (1 CPU: avoid GIL contention), --
    # then upload on a quiet machine
    t0 = time.perf_counter()
    ev = _READY.get(name)
    if ev is not None:
        ev.wait()
    Frun = _RUNNERS.get(name) or _get_runner(name, D0C)
    _EXEC_NS.append(("warmjoin", int((time.perf_counter() - t0) * 1e9)))

    t0 = time.perf_counter()
    B = _get_bass()
    jax = B["jax"]
    dev = jax.devices()[0]
    inF = {"blob": jax.device_put(blob, dev)}
    inF["blob"].block_until_ready()
    _EXEC_NS.append(("puts", int((time.perf_counter() - t0) * 1e9)))

    t0 = time.perf_counter()
    rF = Frun(inF)
    xs_t = np.asarray(rF["xs"])       # [128, 12]
    _EXEC_NS.append(("exec", int((time.perf_counter() - t0) * 1e9)))

    xs = []
    for li in range(L):
        v = np.empty(HID, _f32)
        for c in range(4):
            v[c * 128:(c + 1) * 128] = xs_t[:, li * 4 + c]
        xs.append(v)
    return xs


def total_exec_ns():
    return sum(v for k, v in _EXEC_NS if k == "kernel")


def exec_breakdown():
    return list(_EXEC_NS) + list(_WARM_NS)
